# revision 45
# baseline (speedup 1.0000x reference)
"""Trainium2 Bass kernel for nn_DiffuRNNLayer (B=8, N=2048, D=1024).

Sharding: data-parallel over batch — one batch element per NeuronCore (8 cores).
Per-core kernel works in "layout B" ([d on partitions, n on free]) with the
input pre-transposed on the host.  Phases per core (all intermediates
SBUF-resident, no HBM spills):
  A: Q/K/V projections in fp8 DoubleRow (+elu+1) -> qp/kp/v fp8 SBUF tiles;
     K_sum accumulation.
  B: KV = Kp^T V accumulation straight from SBUF (fp8 DoubleRow, 2 e-halves);
     kv_sb kept as fp8 with a 1/4 scale.
  C: acc = dwconv''(x) + MLP(x) + tokenmixer(LN(x)) into a resident f32 acc
     (diffusion residual and constant per-channel biases folded into the
     dwconv'' taps host-side; tokenmixer gamma folded into the conv1 taps).
  D: attn numerator (fp8 DoubleRow) with C1/norm folded into Qp, acc += attn;
     LN1 (gamma folded into ff_w1); FFN residual; LN2; write y^T (bf16).
Host transposes x/weights in, and the output back out.
"""

import numpy as np
import ml_dtypes
from contextlib import ExitStack

import concourse.bass as bass
import concourse.bacc as bacc
import concourse.tile as tile
import concourse.mybir as mybir
from concourse.bass_utils import run_bass_kernel_spmd

F32 = mybir.dt.float32
BF16 = mybir.dt.bfloat16
FP8 = mybir.dt.float8e4
AF = mybir.ActivationFunctionType
OP = mybir.AluOpType
DR = mybir.MatmulPerfMode.DoubleRow
BF16_NP = ml_dtypes.bfloat16
FP8_NP = ml_dtypes.float8_e4m3

P = 128
D = 1024
DO = D // P  # 8 chunks of the channel dim

KV_SCALE = 0.25          # kv_sb stored as KV/4 in fp8
KSUM_SCALE = 1.0 / 16.0  # ksum stored as ksum/16 in fp8
C1 = float(2.0 ** 21)    # qp fold: qp * C1/norm stays O(1) for fp8

# pp param-plane indices (per-partition params, laid out [128, DO, NP])
(C0, C1i, C2, CB, T0, T1, T2, TCB1, U0, U1, U2,
 TMG, TMB, N1G, N1B, N2G, N2B, LUB1, FFB1, FFB2) = range(20)
NPARAM = 20


def build_nc(N=2048, NT=512, use_bq=False, use_bk=False, use_bv=False,
             use_tmb=False, use_n1b=False, use_n2b=False,
             use_n1g=False, use_n2g=False, debug=False):
    NTILES = N // NT
    NCH = NT // P          # 128-token chunks per tile
    TOTCH = N // P
    W = NT + 4             # phase-C tile width with +-2 halo
    assert N % NT == 0 and NT % P == 0

    nc = bacc.Bacc(None, target_bir_lowering=False, debug=debug)

    xT_d = nc.dram_tensor("x_T", [D, N], BF16, kind="ExternalInput")
    x8_d = nc.dram_tensor("x8", [D, N], FP8, kind="ExternalInput")
    w_d = {}
    for name in ("wqT", "wkT", "wvT", "w1T"):
        w_d[name] = nc.dram_tensor(name, [D, D], FP8, kind="ExternalInput")
    for name in ("w2T", "f1T", "f2T"):
        w_d[name] = nc.dram_tensor(name, [D, D], BF16, kind="ExternalInput")
    pp_d = nc.dram_tensor("pp", [P, DO, NPARAM], F32, kind="ExternalInput")
    diags_d = nc.dram_tensor("diags", [P, 5, DO, P], BF16, kind="ExternalInput")
    rows_d = nc.dram_tensor("rows", [1, 3 * D], BF16, kind="ExternalInput")
    yT_d = nc.dram_tensor("y_T", [D, N], BF16, kind="ExternalOutput")
    qp_sp = nc.dram_tensor("qp_sp", [D, N], FP8)
    qp_r = qp_sp.rearrange("(o p) n -> p o n", p=P)

    xT = xT_d.rearrange("(o p) n -> p o n", p=P)
    x8r = x8_d.rearrange("(o p) n -> p o n", p=P)
    wr = {k: v.rearrange("(o p) n -> p o n", p=P) for k, v in w_d.items()}
    yT = yT_d.rearrange("(o p) n -> p o n", p=P)

    with tile.TileContext(nc) as tc, ExitStack() as top:
        persist = top.enter_context(tc.tile_pool(name="persist", bufs=1))
        pp = persist.tile([P, DO, NPARAM], F32)
        rows = ones_row = ones_1p_bf = None
        if use_bq or use_bk or use_bv:
            rows = persist.tile([1, 3 * D], BF16)
            nc.sync.dma_start(rows, rows_d[:])
            ones_row = persist.tile([1, NT], BF16)
            nc.vector.memset(ones_row, 1.0)
            ones_1p_bf = persist.tile([1, P], BF16)
            nc.vector.memset(ones_1p_bf, 1.0)
        repc_row = persist.tile([1, P], BF16)
        nc.vector.memset(repc_row, C1 * KSUM_SCALE)
        ones_col = persist.tile([P, 1], FP8)
        nc.vector.memset(ones_col, 1.0)
        ones8 = persist.tile([P, 2, P], FP8)
        nc.vector.memset(ones8, 1.0)
        ones_one = persist.tile([1, 1], BF16)
        nc.vector.memset(ones_one, 1.0)
        ksrow_sb = persist.tile([1, D], BF16)
        onesD_bf = persist.tile([P, P], BF16)
        nc.vector.memset(onesD_bf, 1.0 / D)
        eps_ln = persist.tile([P, 1], F32)
        nc.vector.memset(eps_ln, 1e-5)
        kv_sb = persist.tile([P, DO, D], FP8)
        ksum_sb = persist.tile([P, DO, 1], FP8)
        diags = persist.tile([P, 5, DO, P], BF16)

        def stats_mm(psum, lhs_ones, rhs3, width):
            """Accumulate over DO k-chunks: psum[:, j] = sum over channel dim
            times lhs value, replicated across partitions.  rhs3: [P, DO, w]."""
            for c0 in range(0, width, 512):
                cw = min(512, width - c0)
                for kc in range(DO):
                    nc.tensor.matmul(psum[:, c0:c0 + cw], lhs_ones,
                                     rhs3[:, kc, c0:c0 + cw],
                                     start=(kc == 0), stop=(kc == DO - 1))

        def stats_mm8(psum, rhs3, width):
            """fp8 DoubleRow stats: psum[:, j] = SUM over channels (ones=1.0;
            divide by D at consumption).  rhs3: [P, DO, width] fp8."""
            for c0 in range(0, width, 512):
                cw = min(512, width - c0)
                for kp in range(0, DO, 2):
                    nc.tensor.matmul(psum[:, c0:c0 + cw], ones8[:, :, 0:P],
                                     rhs3[:, kp:kp + 2, c0:c0 + cw],
                                     start=(kp == 0), stop=(kp == DO - 2),
                                     perf_mode=DR)

        # Phase-C pools and the resident acc created up-front (LIFO pool
        # order: they outlive kvres); their DMAs are issued between A and B.
        cd = top.enter_context(ExitStack())
        accpool = cd.enter_context(tc.tile_pool(name="accres", bufs=1))
        acc_full = accpool.tile([P, DO, N], F32)
        phC = ExitStack()
        wpoolC = phC.enter_context(tc.tile_pool(name="wC", bufs=1))
        ioC = phC.enter_context(tc.tile_pool(name="ioC", bufs=2))

        # ============ Phases A+B (kp/v SBUF-resident) ============
        with ExitStack() as ph:
            kvres = ph.enter_context(tc.tile_pool(name="kvres", bufs=1))
            kp_full = kvres.tile([P, TOTCH, D], FP8)
            v_full = kvres.tile([P, TOTCH, D], FP8)

            # ---------------- Phase A: QKV (fp8 DoubleRow) ----------------
            # Q-chunks and K/V-chunks are interleaved so the DVE-paced elu
            # chains of one overlap the PE-paced matmuls of the other.
            with ExitStack() as pha:
                wpool = pha.enter_context(tc.tile_pool(name="wA", bufs=1))
                io = pha.enter_context(tc.tile_pool(name="ioA", bufs=2))
                ev = pha.enter_context(tc.tile_pool(name="evA", bufs=2))
                wq_sb = wpool.tile([P, DO, D], FP8, tag="wq")
                nc.sync.dma_start(wq_sb, wr["wqT"])
                xts = {}

                def load_x8(it):
                    t = io.tile([P, DO, NT], FP8, tag="xA", name=f"x8_{it}")
                    nc.sync.dma_start(t, x8r[:, :, it * NT:(it + 1) * NT])
                    xts[it] = t

                load_x8(0)
                wk_sb = wpool.tile([P, DO, D], FP8, tag="wk")
                nc.sync.dma_start(wk_sb, wr["wkT"])
                wv_sb = wpool.tile([P, DO, D], FP8, tag="wv")
                nc.sync.dma_start(wv_sb, wr["wvT"])
                psq_pool = pha.enter_context(tc.tile_pool(name="psAq", bufs=3, space="PSUM"))
                ps = pha.enter_context(tc.tile_pool(name="psA", bufs=2, space="PSUM"))
                ksp = pha.enter_context(tc.tile_pool(name="ksA", bufs=1, space="PSUM"))
                ps_ks = ksp.tile([1, D], F32, tag="ksrow")

                def q_chunk(it, x_t, dc):
                    n0 = it * NT
                    ps_q = psq_pool.tile([P, NT], F32, tag="psq")
                    for kp in range(0, DO, 2):
                        nc.tensor.matmul(ps_q, wq_sb[:, kp:kp + 2, dc * P:(dc + 1) * P],
                                         x_t[:, kp:kp + 2, :], start=(kp == 0),
                                         stop=(kp == DO - 2 and not use_bq),
                                         perf_mode=DR)
                    if use_bq:
                        nc.tensor.matmul(ps_q, rows[0:1, dc * P:(dc + 1) * P],
                                         ones_row[0:1, :], start=False, stop=True)
                    m_t = ev.tile([P, NT], BF16, tag="mA")
                    nc.vector.tensor_scalar_min(m_t, ps_q, 0.0)
                    e_t = ev.tile([P, NT], BF16, tag="eA")
                    nc.scalar.activation(e_t, m_t, AF.Exp)
                    # elu(q)+1 = max(q,0) + exp(min(q,0)); spill chunk to HBM
                    qp_c = ev.tile([P, NT], FP8, tag="qpA")
                    nc.vector.scalar_tensor_tensor(qp_c, ps_q, 0.0, e_t,
                                                   OP.max, OP.add)
                    nc.sync.dma_start(qp_r[:, dc, n0:n0 + NT], qp_c)

                def kv_chunk(it, x_t, ch, half):
                    cg = it * NCH + ch
                    cs = slice(ch * P, (ch + 1) * P)
                    hs = slice(half * 512, (half + 1) * 512)
                    ps_k = ps.tile([P, 512], F32, tag="pskv")
                    for kp in range(0, DO, 2):
                        nc.tensor.matmul(ps_k, x_t[:, kp:kp + 2, cs],
                                         wk_sb[:, kp:kp + 2, hs],
                                         start=(kp == 0),
                                         stop=(kp == DO - 2 and not use_bk),
                                         perf_mode=DR)
                    if use_bk:
                        nc.tensor.matmul(ps_k, ones_1p_bf[0:1, :],
                                         rows[0:1, D + half * 512:D + (half + 1) * 512],
                                         start=False, stop=True)
                    m2 = ev.tile([P, 512], BF16, tag="mA2")
                    nc.vector.tensor_scalar_min(m2, ps_k, 0.0)
                    e2 = ev.tile([P, 512], BF16, tag="eA2")
                    nc.scalar.activation(e2, m2, AF.Exp)
                    nc.vector.scalar_tensor_tensor(kp_full[:, cg, hs],
                                                   ps_k, 0.0,
                                                   e2, OP.max, OP.add)

                    ps_v = ps.tile([P, 512], F32, tag="pskv")
                    for kp in range(0, DO, 2):
                        nc.tensor.matmul(ps_v, x_t[:, kp:kp + 2, cs],
                                         wv_sb[:, kp:kp + 2, hs],
                                         start=(kp == 0),
                                         stop=(kp == DO - 2 and not use_bv),
                                         perf_mode=DR)
                    if use_bv:
                        nc.tensor.matmul(ps_v, ones_1p_bf[0:1, :],
                                         rows[0:1, 2 * D + half * 512:2 * D + (half + 1) * 512],
                                         start=False, stop=True)
                    nc.scalar.activation(v_full[:, cg, hs], ps_v, AF.Copy)
                    # K_sum accumulation as a [1, D] row (tokens on partitions)
                    nc.tensor.matmul(ps_ks[0:1, hs], ones_col,
                                     kp_full[:, cg, hs],
                                     start=(cg == 0),
                                     stop=(cg == TOTCH - 1))

                for it in range(NTILES):
                    if it + 1 < NTILES:
                        load_x8(it + 1)
                    x_t = xts.pop(it)
                    for i in range(DO):
                        q_chunk(it, x_t, i)
                        kv_chunk(it, x_t, i // 2, i % 2)
                nc.scalar.activation(ksrow_sb, ps_ks[0:1, :], AF.Copy)
                # transpose K_sum row -> per-partition column layout [P, DO]
                ps_ksc = ksp.tile([P, DO], F32, tag="kscol")
                for dc in range(DO):
                    nc.tensor.matmul(ps_ksc[:, dc:dc + 1],
                                     ksrow_sb[0:1, dc * P:(dc + 1) * P],
                                     ones_one[0:1, 0:1], start=True, stop=True)
                nc.scalar.activation(ksum_sb[:, :, 0], ps_ksc, AF.Copy,
                                     scale=KSUM_SCALE)

            # ---- hoisted phase-C prefetches (overlap with B) ----
            nc.sync.dma_start(pp, pp_d[:])
            nc.sync.dma_start(diags, diags_d[:])
            w1_sb = wpoolC.tile([P, DO, D], FP8, tag="w1")
            nc.sync.dma_start(w1_sb, wr["w1T"])
            w2_sb = wpoolC.tile([P, DO, D], BF16, tag="w2")
            nc.sync.dma_start(w2_sb, wr["w2T"])
            xc_tiles = {}

            def load_xc(it):
                n0 = it * NT
                x_t = ioC.tile([P, DO, W], BF16, tag="xC", name=f"x_{it}")
                x8_t = ioC.tile([P, DO, W], FP8, tag="x8C", name=f"x8c_{it}")
                lo, hi = n0 - 2, n0 + NT + 2
                if lo < 0:
                    nc.vector.memset(x_t[:, :, 0:2], 0.0)
                    nc.sync.dma_start(x_t[:, :, 2:W], xT[:, :, 0:hi])
                    nc.vector.memset(x8_t[:, :, 0:2], 0.0)
                    nc.sync.dma_start(x8_t[:, :, 2:W], x8r[:, :, 0:hi])
                elif hi > N:
                    nc.vector.memset(x_t[:, :, W - 2:W], 0.0)
                    nc.sync.dma_start(x_t[:, :, 0:W - 2], xT[:, :, lo:N])
                    nc.vector.memset(x8_t[:, :, W - 2:W], 0.0)
                    nc.sync.dma_start(x8_t[:, :, 0:W - 2], x8r[:, :, lo:N])
                else:
                    nc.sync.dma_start(x_t, xT[:, :, lo:hi])
                    nc.sync.dma_start(x8_t, x8r[:, :, lo:hi])
                xc_tiles[it] = (x_t, x8_t)

            load_xc(0)

            # ------------ Phase B: KV accumulation (fp8 DR, SBUF src) ------------
            with ExitStack() as phb:
                psb = phb.enter_context(tc.tile_pool(name="psB", bufs=1, space="PSUM"))
                for eh in range(D // 512):
                    hs = slice(eh * 512, (eh + 1) * 512)
                    kv_ps = [psb.tile([P, 512], F32, tag=f"kvps{d}", name=f"kvps{d}_{eh}")
                             for d in range(DO)]
                    for chp in range(0, TOTCH, 2):
                        for dc in range(DO):
                            nc.tensor.matmul(kv_ps[dc],
                                             kp_full[:, chp:chp + 2, dc * P:(dc + 1) * P],
                                             v_full[:, chp:chp + 2, hs],
                                             start=(chp == 0),
                                             stop=(chp == TOTCH - 2), perf_mode=DR)
                    for dc in range(DO):
                        nc.scalar.activation(kv_sb[:, dc, hs], kv_ps[dc], AF.Copy,
                                             scale=KV_SCALE)

        # ---------------- Phase C: conv'' + local MLP + token mixer ----------------
        with phC as ph:
            pipe = ph.enter_context(tc.tile_pool(name="pipeC", bufs=2))
            mid = ph.enter_context(tc.tile_pool(name="midC", bufs=1))
            sm = ph.enter_context(tc.tile_pool(name="smC", bufs=1))
            ps = ph.enter_context(tc.tile_pool(name="psC", bufs=2, space="PSUM"))
            pst = ph.enter_context(tc.tile_pool(name="pstC", bufs=1, space="PSUM"))

            def c_front(it):
                n0 = it * NT
                x_t, x8_t = xc_tiles.pop(it)
                acc = acc_full[:, :, n0:n0 + NT]
                # diffusion dwconv'' center tap + bias on ACT (side taps join
                # the c_back PSUM group as diagonal matmuls)
                for o in range(DO):
                    nc.scalar.activation(acc[:, o, :], x_t[:, o, 2:NT + 2],
                                         AF.Identity, bias=pp[:, o, CB:CB + 1],
                                         scale=pp[:, o, C1i:C1i + 1])

                # local MLP first half (fp8 DoubleRow; w1 pre-scaled x16)
                h1_t = pipe.tile([P, DO, NT], BF16, tag="h1", name=f"h1_{it}")
                for dc in range(DO):
                    ps_h = ps.tile([P, NT], F32, tag="psh1", name=f"psh1_{it}_{dc}")
                    for kp in range(0, DO, 2):
                        nc.tensor.matmul(ps_h, w1_sb[:, kp:kp + 2, dc * P:(dc + 1) * P],
                                         x8_t[:, kp:kp + 2, 2:NT + 2],
                                         start=(kp == 0), stop=(kp == DO - 2),
                                         perf_mode=DR)
                    nc.scalar.activation(h1_t[:, dc, :], ps_h, AF.Gelu,
                                         bias=pp[:, dc, LUB1:LUB1 + 1],
                                         scale=1.0 / 16.0)

                # token mixer LN stats via fp8 DR on x8 (ones=1; /D at readout)
                sq_t = mid.tile([P, DO, W], FP8, tag="sq8", name=f"sq_{it}")
                nc.scalar.activation(sq_t, x8_t, AF.Square)
                ps_m = pst.tile([P, W], F32, tag="psm", name=f"psm_{it}")
                stats_mm8(ps_m, x8_t, W)
                ps_s = pst.tile([P, W], F32, tag="pss", name=f"pss_{it}")
                stats_mm8(ps_s, sq_t, W)
                m_sb = sm.tile([P, W], BF16, tag="msb", name=f"msb_{it}")
                nc.scalar.activation(m_sb, ps_m, AF.Copy, scale=1.0 / D)
                var = sm.tile([P, W], F32, tag="var", name=f"var_{it}")
                nc.scalar.activation(var, ps_m, AF.Square, scale=1.0 / D)
                nc.vector.scalar_tensor_tensor(var, ps_s, 1.0 / D, var,
                                               OP.mult, OP.subtract)
                nc.scalar.activation(var, var, AF.Sqrt, bias=eps_ln[:, 0:1])
                nc.vector.reciprocal_approx_fast(out=var, in_=var)
                rstd = sm.tile([P, W], BF16, tag="rstd", name=f"rstd_{it}")
                nc.vector.tensor_copy(rstd, var)
                # xm = (x - m) * rstd  (tm gamma folded into conv1 taps)
                xm_t = mid.tile([P, DO, W], BF16, tag="tokC", name=f"xm_{it}")
                nc.vector.tensor_sub(xm_t, x_t,
                                     m_sb[:, None, :].broadcast_to([P, DO, W]))
                nc.vector.tensor_mul(xm_t, xm_t,
                                     rstd[:, None, :].broadcast_to([P, DO, W]))
                if use_tmb:
                    for o in range(DO):
                        nc.vector.tensor_scalar_add(xm_t[:, o, :], xm_t[:, o, :],
                                                    pp[:, o, TMB:TMB + 1])
                # conv1: t_s[k] = conv1(xm)[k+1], k in [0, W-2)
                t_t = mid.tile([P, DO, W - 2], BF16, tag="tokD", name=f"t_{it}")
                for o in range(DO):
                    nc.scalar.activation(t_t[:, o, :], xm_t[:, o, 1:W - 1],
                                         AF.Identity, bias=pp[:, o, TCB1:TCB1 + 1],
                                         scale=pp[:, o, T1:T1 + 1])
                for o in range(DO):
                    nc.vector.scalar_tensor_tensor(t_t[:, o, :], xm_t[:, o, 0:W - 2],
                                                   pp[:, o, T0:T0 + 1],
                                                   t_t[:, o, :], OP.mult, OP.add)
                for o in range(DO):
                    nc.vector.scalar_tensor_tensor(t_t[:, o, :], xm_t[:, o, 2:W],
                                                   pp[:, o, T2:T2 + 1],
                                                   t_t[:, o, :], OP.mult, OP.add)
                t2_t = pipe.tile([P, DO, W - 2], BF16, tag="t2", name=f"t2_{it}")
                nc.scalar.activation(t2_t, t_t, AF.Gelu)
                if it == 0:
                    nc.vector.memset(t2_t[:, :, 0:1], 0.0)
                if it == NTILES - 1:
                    nc.vector.memset(t2_t[:, :, W - 3:W - 2], 0.0)
                return x_t, h1_t, t2_t

            def c_back(it, tiles):
                n0 = it * NT
                x_t, h1_t, t2_t = tiles
                for dc in range(DO):
                    ps_h = ps.tile([P, NT], F32, tag="psh2", name=f"psh2_{it}_{dc}")
                    for kc in range(DO):
                        nc.tensor.matmul(ps_h, w2_sb[:, kc, dc * P:(dc + 1) * P],
                                         h1_t[:, kc, :],
                                         start=(kc == 0), stop=False)
                    for tap in range(3):
                        nc.tensor.matmul(ps_h, diags[:, tap, dc, :],
                                         t2_t[:, dc, tap:NT + tap],
                                         start=False, stop=False)
                    # diffusion conv side taps as diagonal matmuls
                    nc.tensor.matmul(ps_h, diags[:, 3, dc, :],
                                     x_t[:, dc, 1:NT + 1], start=False, stop=False)
                    nc.tensor.matmul(ps_h, diags[:, 4, dc, :],
                                     x_t[:, dc, 3:NT + 3], start=False, stop=True)
                    nc.vector.tensor_add(acc_full[:, dc, n0:n0 + NT],
                                         acc_full[:, dc, n0:n0 + NT], ps_h)

            pend = {0: c_front(0)}
            for it in range(NTILES):
                if it + 1 < NTILES:
                    load_xc(it + 1)
                    pend[it + 1] = c_front(it + 1)
                c_back(it, pend.pop(it))

        # ---------------- Phase D: attention + LN1 + FFN + LN2 ----------------
        with ExitStack() as ph:
            wpoolD = ph.enter_context(tc.tile_pool(name="wD", bufs=1))
            f1_sb = wpoolD.tile([P, DO, D], BF16, tag="f1")
            nc.sync.dma_start(f1_sb, wr["f1T"])
            f2_sb = wpoolD.tile([P, DO, D], BF16, tag="f2")
            nc.sync.dma_start(f2_sb, wr["f2T"])
            ioD = ph.enter_context(tc.tile_pool(name="ioD", bufs=2))
            mid = ph.enter_context(tc.tile_pool(name="midD", bufs=1))
            sm = ph.enter_context(tc.tile_pool(name="smD", bufs=2))
            ps = ph.enter_context(tc.tile_pool(name="psD", bufs=2, space="PSUM"))
            psf_pool = ph.enter_context(tc.tile_pool(name="psfD", bufs=2, space="PSUM"))
            pst = ph.enter_context(tc.tile_pool(name="pstD", bufs=1, space="PSUM"))

            def d_front_a(it):
                """norm row, C1/norm fold, numerator halves 0-3."""
                n0 = it * NT
                qp_t = ioD.tile([P, DO, NT], FP8, tag="qpD", name=f"qp_{it}")
                nc.sync.dma_start(qp_t, qp_r[:, :, n0:n0 + NT])
                acc_t = acc_full[:, :, n0:n0 + NT]
                ps_n = pst.tile([P, NT], F32, tag="psrep", name=f"psn_{it}")
                for kc in range(DO):
                    nc.tensor.matmul(ps_n[0:1, :], ksum_sb[:, kc, :],
                                     qp_t[:, kc, :],
                                     start=(kc == 0), stop=(kc == DO - 1))
                nr = sm.tile([1, NT], F32, tag="nrD", name=f"nr_{it}")
                nc.vector.tensor_scalar_add(nr, ps_n[0:1, :], 1e-6 * KSUM_SCALE)
                rr = sm.tile([1, NT], F32, tag="rrD", name=f"rr_{it}")
                nc.vector.reciprocal_approx_fast(out=rr, in_=nr)
                rrb = sm.tile([1, NT], BF16, tag="rrbD", name=f"rrb_{it}")
                nc.vector.tensor_copy(rrb, rr)
                # rep = C1*KSUM_SCALE * (1/(norm*KSUM_SCALE)) = C1/norm
                ps_rep = pst.tile([P, NT], F32, tag="psrep", name=f"psrep_{it}")
                nc.tensor.matmul(ps_rep, repc_row[0:1, :], rrb, start=True,
                                 stop=True)
                rep_sb = mid.tile([P, NT], BF16, tag="repsb", name=f"rep_{it}")
                nc.scalar.activation(rep_sb, ps_rep, AF.Copy)
                nc.vector.tensor_mul(qp_t, qp_t,
                                     rep_sb[:, None, :].broadcast_to([P, DO, NT]))
                for ec in range(DO // 2):
                    ps_u = ps.tile([P, NT], F32, tag="psnum", name=f"psnum_{it}_{ec}")
                    for kp in range(0, DO, 2):
                        nc.tensor.matmul(ps_u, kv_sb[:, kp:kp + 2, ec * P:(ec + 1) * P],
                                         qp_t[:, kp:kp + 2, :],
                                         start=(kp == 0), stop=(kp == DO - 2),
                                         perf_mode=DR)
                    nc.vector.scalar_tensor_tensor(acc_t[:, ec, :], ps_u,
                                                   1.0 / (KV_SCALE * C1),
                                                   acc_t[:, ec, :], OP.mult, OP.add)
                return qp_t, acc_t

            def d_front_b(it, T):
                qp_t, acc_t = T
                for ec in range(DO // 2, DO):
                    ps_u = ps.tile([P, NT], F32, tag="psnum", name=f"psnum_{it}_{ec}")
                    for kp in range(0, DO, 2):
                        nc.tensor.matmul(ps_u, kv_sb[:, kp:kp + 2, ec * P:(ec + 1) * P],
                                         qp_t[:, kp:kp + 2, :],
                                         start=(kp == 0), stop=(kp == DO - 2),
                                         perf_mode=DR)
                    nc.vector.scalar_tensor_tensor(acc_t[:, ec, :], ps_u,
                                                   1.0 / (KV_SCALE * C1),
                                                   acc_t[:, ec, :], OP.mult, OP.add)
                return acc_t

            def d_mid(it, acc_t):
                """LN1 stats + apply -> y1 (bf16; n1 gamma folded into f1)."""
                acc_bf = mid.tile([P, DO, NT], BF16, tag="accbf", name=f"accbf_{it}")
                nc.scalar.activation(acc_bf, acc_t, AF.Copy)
                sq_t = mid.tile([P, DO, NT], FP8, tag="sqD8", name=f"sqD_{it}")
                nc.scalar.activation(sq_t, acc_t, AF.Square)
                ps_m1 = pst.tile([P, NT], F32, tag="psm1", name=f"psm1_{it}")
                stats_mm(ps_m1, onesD_bf, acc_bf, NT)
                ps_s1 = pst.tile([P, NT], F32, tag="pss1", name=f"pss1_{it}")
                stats_mm8(ps_s1, sq_t, NT)
                m1_sb = sm.tile([P, NT], BF16, tag="m1sb", name=f"m1_{it}")
                nc.scalar.activation(m1_sb, ps_m1, AF.Copy)
                var1 = sm.tile([P, NT], F32, tag="varD", name=f"var1_{it}")
                nc.scalar.activation(var1, ps_m1, AF.Square)
                nc.vector.scalar_tensor_tensor(var1, ps_s1, 1.0 / D, var1,
                                               OP.mult, OP.subtract)
                nc.scalar.activation(var1, var1, AF.Sqrt, bias=eps_ln[:, 0:1])
                nc.vector.reciprocal_approx_fast(out=var1, in_=var1)
                rstd1 = sm.tile([P, NT], BF16, tag="rstdb", name=f"rstdb_{it}")
                nc.vector.tensor_copy(rstd1, var1)
                y1_t = mid.tile([P, DO, NT], BF16, tag="y1", name=f"y1_{it}")
                nc.vector.tensor_sub(y1_t, acc_bf,
                                     m1_sb[:, None, :].broadcast_to([P, DO, NT]))
                nc.vector.tensor_mul(y1_t, y1_t,
                                     rstd1[:, None, :].broadcast_to([P, DO, NT]))
                if use_n1b:
                    for o in range(DO):
                        nc.vector.tensor_scalar_add(y1_t[:, o, :], y1_t[:, o, :],
                                                    pp[:, o, N1B:N1B + 1])
                return y1_t

            def d_ffn(it, y1_t):
                f1h_t = mid.tile([P, DO, NT], BF16, tag="f1h", name=f"f1h_{it}")
                for dc in range(DO):
                    ps_f = psf_pool.tile([P, NT], F32, tag="psf",
                                         name=f"psf1_{it}_{dc}")
                    for kc in range(DO):
                        nc.tensor.matmul(ps_f, f1_sb[:, kc, dc * P:(dc + 1) * P],
                                         y1_t[:, kc, :],
                                         start=(kc == 0), stop=(kc == DO - 1))
                    nc.scalar.activation(f1h_t[:, dc, :], ps_f, AF.Gelu,
                                         bias=pp[:, dc, FFB1:FFB1 + 1])
                y2_t = mid.tile([P, DO, NT], BF16, tag="y2", name=f"y2_{it}")
                for dc in range(DO):
                    ps_f = psf_pool.tile([P, NT], F32, tag="psf",
                                         name=f"psf2_{it}_{dc}")
                    for kc in range(DO):
                        nc.tensor.matmul(ps_f, f2_sb[:, kc, dc * P:(dc + 1) * P],
                                         f1h_t[:, kc, :],
                                         start=(kc == 0), stop=(kc == DO - 1))
                    if use_n1g:
                        # y2 = y1*g + f2(h); (residual gamma must be re-applied)
                        nc.vector.scalar_tensor_tensor(y2_t[:, dc, :],
                                                       y1_t[:, dc, :],
                                                       pp[:, dc, N1G:N1G + 1],
                                                       ps_f, OP.mult, OP.add)
                    else:
                        nc.vector.scalar_tensor_tensor(y2_t[:, dc, :], ps_f,
                                                       pp[:, dc, FFB2:FFB2 + 1],
                                                       y1_t[:, dc, :], OP.add, OP.add)
                return y2_t

            def d_back(it, y2_t, c0=0, cw=None):
                if cw is None:
                    cw = NT
                n0 = it * NT + c0
                y2s = y2_t[:, :, c0:c0 + cw]
                sq2_t = mid.tile([P, DO, NT], FP8, tag="sq28", name=f"sq2_{it}_{c0}")
                nc.scalar.activation(sq2_t[:, :, 0:cw], y2s, AF.Square)
                ps_m2 = pst.tile([P, NT], F32, tag="psm2", name=f"psm2_{it}_{c0}")
                stats_mm(ps_m2[:, 0:cw], onesD_bf, y2s, cw)
                ps_s2 = pst.tile([P, NT], F32, tag="pss1", name=f"pss2_{it}_{c0}")
                stats_mm8(ps_s2[:, 0:cw], sq2_t[:, :, 0:cw], cw)
                m2_sb = sm.tile([P, NT], BF16, tag="m2sb", name=f"m2_{it}_{c0}")
                nc.scalar.activation(m2_sb[:, 0:cw], ps_m2[:, 0:cw], AF.Copy)
                var2 = sm.tile([P, NT], F32, tag="varD", name=f"var2_{it}_{c0}")
                nc.scalar.activation(var2[:, 0:cw], ps_m2[:, 0:cw], AF.Square)
                nc.vector.scalar_tensor_tensor(var2[:, 0:cw], ps_s2[:, 0:cw],
                                               1.0 / D, var2[:, 0:cw],
                                               OP.mult, OP.subtract)
                nc.scalar.activation(var2[:, 0:cw], var2[:, 0:cw], AF.Sqrt,
                                     bias=eps_ln[:, 0:1])
                nc.vector.reciprocal_approx_fast(out=var2[:, 0:cw],
                                                 in_=var2[:, 0:cw])
                rstd2 = sm.tile([P, NT], BF16, tag="rstd2", name=f"rstd2_{it}_{c0}")
                nc.vector.tensor_copy(rstd2[:, 0:cw], var2[:, 0:cw])
                yo_t = mid.tile([P, DO, NT], BF16, tag="yo", name=f"yo_{it}_{c0}")
                yo = yo_t[:, :, 0:cw]
                nc.vector.tensor_sub(yo, y2s,
                                     m2_sb[:, None, 0:cw].broadcast_to([P, DO, cw]))
                if use_n2g:
                    for o in range(DO):
                        nc.vector.scalar_tensor_tensor(yo[:, o, :], yo[:, o, :],
                                                       pp[:, o, N2G:N2G + 1],
                                                       rstd2[:, 0:cw],
                                                       OP.mult, OP.mult)
                else:
                    nc.vector.tensor_mul(yo, yo,
                                         rstd2[:, None, 0:cw].broadcast_to([P, DO, cw]))
                if use_n2b:
                    for o in range(DO):
                        nc.vector.tensor_scalar_add(yo[:, o, :], yo[:, o, :],
                                                    pp[:, o, N2B:N2B + 1])
                nc.sync.dma_start(yT[:, :, n0:n0 + cw], yo)

            # Pipeline: tile t+1's numerator halves run under tile t's LN/FFN,
            # and tile t+1's LN1 chain is issued before tile t's LN2 so the
            # FFN matmuls of t+1 can flow while t's LN2 drains on DVE/ACT.
            acc_cur = d_front_b(0, d_front_a(0))
            y1_cur = d_mid(0, acc_cur)
            for it in range(NTILES):
                Tnext = d_front_a(it + 1) if it + 1 < NTILES else None
                y2_cur = d_ffn(it, y1_cur)
                if Tnext is not None:
                    acc_next = d_front_b(it + 1, Tnext)
                    y1_cur = d_mid(it + 1, acc_next)
                    d_back(it, y2_cur)
                else:
                    # last tile: split LN2 so the final chain+DMA pipelines
                    d_back(it, y2_cur, 0, NT // 2)
                    d_back(it, y2_cur, NT // 2, NT // 2)

    nc.compile()
    return nc


def make_in_maps(inputs, n_cores=8):
    """Host-side preprocessing: fold constants, transpose, cast, shard."""
    x = np.asarray(inputs["x"], np.float32)
    B, N, D_ = x.shape
    dt = float(np.asarray(inputs["delta_t"]))

    def g(k):
        return np.asarray(inputs[k], np.float32)

    diff_w, diff_b = g("diff_w"), g("diff_b")
    tm_w1, tm_cb1 = g("tm_w1"), g("tm_cb1")
    tm_w2, tm_cb2 = g("tm_w2"), g("tm_cb2")
    tm_g = g("tm_g")
    n1_g, n2_g = g("n1_g"), g("n2_g")

    pp = np.zeros((P, DO, NPARAM), np.float32)

    def put(i, v):
        pp[:, :, i] = v.reshape(DO, P).T

    put(C0, dt * diff_w[:, 0, 0])
    put(C1i, dt * diff_w[:, 0, 1] + (1.0 - dt))
    put(C2, dt * diff_w[:, 0, 2])
    put(CB, dt * diff_b + g("lu_b2") + tm_cb2)
    # token-mixer gamma folded into the conv1 taps
    put(T0, tm_w1[:, 0, 0] * tm_g)
    put(T1, tm_w1[:, 0, 1] * tm_g)
    put(T2, tm_w1[:, 0, 2] * tm_g)
    put(TCB1, tm_cb1)
    put(U0, tm_w2[:, 0, 0])
    put(U1, tm_w2[:, 0, 1])
    put(U2, tm_w2[:, 0, 2])
    put(TMG, tm_g)
    put(TMB, g("tm_beta"))
    put(N1G, n1_g)
    put(N1B, g("n1_b"))
    put(N2G, n2_g)
    put(N2B, g("n2_b"))
    put(LUB1, g("lu_b1"))
    put(FFB1, g("ff_b1"))
    put(FFB2, g("ff_b2"))

    # taps 0-2: token-mixer conv2; taps 3-4: diffusion conv side taps
    diags = np.zeros((P, 5, DO, P), np.float32)
    idx = np.arange(P)
    for tap in range(3):
        for dc in range(DO):
            diags[idx, tap, dc, idx] = tm_w2[dc * P + idx, 0, tap]
    for dc in range(DO):
        diags[idx, 3, dc, idx] = dt * diff_w[dc * P + idx, 0, 0]
        diags[idx, 4, dc, idx] = dt * diff_w[dc * P + idx, 0, 2]
    diags = diags.astype(BF16_NP)

    rows = np.zeros((1, 3 * D), np.float32)
    rows[0, 0:D] = g("bq")
    rows[0, D:2 * D] = g("bk")
    rows[0, 2 * D:3 * D] = g("bv")
    rows = rows.astype(BF16_NP)

    use_n1g = bool(np.any(n1_g != 1.0))
    use_n2g = bool(np.any(n2_g != 1.0))

    wt = {}
    for name, key in (("w2T", "lu_w2"), ("f2T", "ff_w2")):
        wt[name] = np.ascontiguousarray(g(key).T).astype(BF16_NP)
    # n1 gamma folded into ff_w1 input rows (when gamma != 1)
    f1 = g("ff_w1")
    if use_n1g:
        f1 = f1 * n1_g[None, :]
    wt["f1T"] = np.ascontiguousarray(f1.T).astype(BF16_NP)
    # w1 shipped as fp8 pre-scaled x16 (consumer applies 1/16 via ACT scale)
    wt["w1T"] = np.ascontiguousarray(
        np.clip(g("lu_w1").T * 16.0, -240, 240)).astype(FP8_NP)
    for name, key in (("wqT", "wq"), ("wkT", "wk"), ("wvT", "wv")):
        wt[name] = np.ascontiguousarray(
            np.clip(g(key).T, -240, 240)).astype(FP8_NP)

    xT = np.ascontiguousarray(x.transpose(0, 2, 1)).astype(BF16_NP)
    x8 = np.clip(xT.astype(np.float32), -240, 240).astype(FP8_NP)

    flags = dict(
        use_bq=bool(np.any(g("bq"))),
        use_bk=bool(np.any(g("bk"))),
        use_bv=bool(np.any(g("bv"))),
        use_tmb=bool(np.any(g("tm_beta"))),
        use_n1b=bool(np.any(g("n1_b"))),
        use_n2b=bool(np.any(g("n2_b"))),
        use_n1g=use_n1g,
        use_n2g=use_n2g,
    )

    shared = {**wt, "pp": pp, "rows": rows, "diags": diags}
    in_maps = [{**shared, "x_T": xT[b], "x8": x8[b]} for b in range(n_cores)]
    return in_maps, flags, (B, N)


_NC_CACHE = {}


def kernel(**inputs):
    in_maps, flags, (B, N) = make_in_maps(inputs)
    key = (N, tuple(sorted(flags.items())))
    if key not in _NC_CACHE:
        _NC_CACHE[key] = build_nc(N=N, NT=512, **flags)
    nc = _NC_CACHE[key]
    res = run_bass_kernel_spmd(nc, in_maps, list(range(B)))
    y = np.stack([res.results[b]["y_T"] for b in range(B)])
    return np.ascontiguousarray(y.transpose(0, 2, 1)).astype(np.float32)


# revision 54
# speedup vs baseline: 1.0363x; 1.0363x over previous
"""Trainium2 Bass kernel for nn_DiffuRNNLayer (B=8, N=2048, D=1024).

Sharding: data-parallel over batch — one batch element per NeuronCore (8 cores).
Per-core kernel works in "layout B" ([d on partitions, n on free]) with the
input pre-transposed on the host.  Phases per core (all intermediates
SBUF-resident, no HBM spills):
  A: Q/K/V projections in fp8 DoubleRow (+elu+1) -> qp/kp/v fp8 SBUF tiles;
     K_sum accumulation.
  B: KV = Kp^T V accumulation straight from SBUF (fp8 DoubleRow, 2 e-halves);
     kv_sb kept as fp8 with a 1/4 scale.
  C: acc = dwconv''(x) + MLP(x) + tokenmixer(LN(x)) into a resident f32 acc
     (diffusion residual and constant per-channel biases folded into the
     dwconv'' taps host-side; tokenmixer gamma folded into the conv1 taps).
  D: attn numerator (fp8 DoubleRow) with C1/norm folded into Qp, acc += attn;
     LN1 (gamma folded into ff_w1); FFN residual; LN2; write y^T (bf16).
Host transposes x/weights in, and the output back out.
"""

import numpy as np
import ml_dtypes
from contextlib import ExitStack

import concourse.bass as bass
import concourse.bacc as bacc
import concourse.tile as tile
import concourse.mybir as mybir
from concourse.bass_utils import run_bass_kernel_spmd

F32 = mybir.dt.float32
BF16 = mybir.dt.bfloat16
FP8 = mybir.dt.float8e4
AF = mybir.ActivationFunctionType
OP = mybir.AluOpType
DR = mybir.MatmulPerfMode.DoubleRow
BF16_NP = ml_dtypes.bfloat16
FP8_NP = ml_dtypes.float8_e4m3

P = 128
D = 1024
DO = D // P  # 8 chunks of the channel dim

KV_SCALE = 0.25          # kv_sb stored as KV/4 in fp8
KSUM_SCALE = 1.0 / 16.0  # ksum stored as ksum/16 in fp8
C1 = float(2.0 ** 21)    # qp fold: qp * C1/norm stays O(1) for fp8

# pp param-plane indices (per-partition params, laid out [128, DO, NP])
(C0, C1i, C2, CB, T0, T1, T2, TCB1, U0, U1, U2,
 TMG, TMB, N1G, N1B, N2G, N2B, LUB1, FFB1, FFB2) = range(20)
NPARAM = 20


def build_nc(N=2048, NT=512, use_bq=False, use_bk=False, use_bv=False,
             use_tmb=False, use_n1b=False, use_n2b=False,
             use_n1g=False, use_n2g=False, debug=False):
    NTILES = N // NT
    NCH = NT // P          # 128-token chunks per tile
    TOTCH = N // P
    W = NT + 4             # phase-C tile width with +-2 halo
    W8 = NT + 8            # x8 tile width padded so dual-fp8 LDW strides are 8-aligned
    assert N % NT == 0 and NT % P == 0

    nc = bacc.Bacc(None, target_bir_lowering=False, debug=debug)

    xT_d = nc.dram_tensor("x_T", [D, N], BF16, kind="ExternalInput")
    x8_d = nc.dram_tensor("x8", [D, N], FP8, kind="ExternalInput")
    w_d = {}
    for name in ("wqT", "wkT", "wvT", "w1T"):
        w_d[name] = nc.dram_tensor(name, [D, D], FP8, kind="ExternalInput")
    for name in ("w2T", "f1T", "f2T"):
        w_d[name] = nc.dram_tensor(name, [D, D], BF16, kind="ExternalInput")
    pp_d = nc.dram_tensor("pp", [P, DO, NPARAM], F32, kind="ExternalInput")
    diags_d = nc.dram_tensor("diags", [P, 3, DO, P], FP8, kind="ExternalInput")
    diagsb_d = nc.dram_tensor("diagsb", [P, 2, DO, P], BF16, kind="ExternalInput")
    rows_d = nc.dram_tensor("rows", [1, 3 * D], BF16, kind="ExternalInput")
    yT_d = nc.dram_tensor("y_T", [D, N], BF16, kind="ExternalOutput")
    qp_sp = nc.dram_tensor("qp_sp", [D, N], FP8)
    qp_r = qp_sp.rearrange("(o p) n -> p o n", p=P)

    xT = xT_d.rearrange("(o p) n -> p o n", p=P)
    x8r = x8_d.rearrange("(o p) n -> p o n", p=P)
    wr = {k: v.rearrange("(o p) n -> p o n", p=P) for k, v in w_d.items()}
    yT = yT_d.rearrange("(o p) n -> p o n", p=P)

    with tile.TileContext(nc) as tc, ExitStack() as top:
        persist = top.enter_context(tc.tile_pool(name="persist", bufs=1))
        pp = persist.tile([P, DO, NPARAM], F32)
        rows = ones_row = ones_1p_bf = None
        if use_bq or use_bk or use_bv:
            rows = persist.tile([1, 3 * D], BF16)
            nc.sync.dma_start(rows, rows_d[:])
            ones_row = persist.tile([1, NT], BF16)
            nc.vector.memset(ones_row, 1.0)
            ones_1p_bf = persist.tile([1, P], BF16)
            nc.vector.memset(ones_1p_bf, 1.0)
        repc_row = persist.tile([1, P], BF16)
        nc.vector.memset(repc_row, C1 * KSUM_SCALE)
        ones_col = persist.tile([P, 1], FP8)
        nc.vector.memset(ones_col, 1.0)
        ones8 = persist.tile([P, 2, P], FP8)
        nc.vector.memset(ones8, 1.0)
        ones_one = persist.tile([1, 1], BF16)
        nc.vector.memset(ones_one, 1.0)
        ksrow_sb = persist.tile([1, D], BF16)
        onesD_bf = persist.tile([P, P], BF16)
        nc.vector.memset(onesD_bf, 1.0 / D)
        eps_ln = persist.tile([P, 1], F32)
        nc.vector.memset(eps_ln, 1e-5)
        kv_sb = persist.tile([P, DO, D], FP8)
        ksum_sb = persist.tile([P, DO, 1], FP8)
        diags = persist.tile([P, 3, DO, P], FP8)
        diagsb = persist.tile([P, 2, DO, P], BF16)

        def stats_mm(psum, lhs_ones, rhs3, width):
            """Accumulate over DO k-chunks: psum[:, j] = sum over channel dim
            times lhs value, replicated across partitions.  rhs3: [P, DO, w]."""
            for c0 in range(0, width, 512):
                cw = min(512, width - c0)
                for kc in range(DO):
                    nc.tensor.matmul(psum[:, c0:c0 + cw], lhs_ones,
                                     rhs3[:, kc, c0:c0 + cw],
                                     start=(kc == 0), stop=(kc == DO - 1))

        def stats_mm8(psum, rhs3, width):
            """fp8 DoubleRow stats: psum[:, j] = SUM over channels (ones=1.0;
            divide by D at consumption).  rhs3: [P, DO, width] fp8."""
            for c0 in range(0, width, 512):
                cw = min(512, width - c0)
                for kp in range(0, DO, 2):
                    nc.tensor.matmul(psum[:, c0:c0 + cw], ones8[:, :, 0:P],
                                     rhs3[:, kp:kp + 2, c0:c0 + cw],
                                     start=(kp == 0), stop=(kp == DO - 2),
                                     perf_mode=DR)

        # Phase-C pools and the resident acc created up-front (LIFO pool
        # order: they outlive kvres); their DMAs are issued between A and B.
        cd = top.enter_context(ExitStack())
        accpool = cd.enter_context(tc.tile_pool(name="accres", bufs=1))
        acc_full = accpool.tile([P, DO, N], F32)
        phC = ExitStack()
        wpoolC = phC.enter_context(tc.tile_pool(name="wC", bufs=1))
        ioC = phC.enter_context(tc.tile_pool(name="ioC", bufs=3))
        ioC8 = phC.enter_context(tc.tile_pool(name="ioC8", bufs=2))

        # ============ Phases A+B (kp/v SBUF-resident) ============
        with ExitStack() as ph:
            kvres = ph.enter_context(tc.tile_pool(name="kvres", bufs=1))
            kp_full = kvres.tile([P, TOTCH, D], FP8)
            v_full = kvres.tile([P, TOTCH, D], FP8)

            # ---------------- Phase A: QKV (fp8 DoubleRow) ----------------
            # Q-chunks and K/V-chunks are interleaved so the DVE-paced elu
            # chains of one overlap the PE-paced matmuls of the other.
            with ExitStack() as pha:
                wpool = pha.enter_context(tc.tile_pool(name="wA", bufs=1))
                io = pha.enter_context(tc.tile_pool(name="ioA", bufs=2))
                ev = pha.enter_context(tc.tile_pool(name="evA", bufs=2))
                wq_sb = wpool.tile([P, DO, D], FP8, tag="wq")
                nc.sync.dma_start(wq_sb, wr["wqT"])
                xts = {}

                def load_x8(it):
                    t = io.tile([P, DO, NT], FP8, tag="xA", name=f"x8a_{it}")
                    nc.sync.dma_start(t, x8r[:, :, it * NT:(it + 1) * NT])
                    xts[it] = t

                load_x8(0)
                wk_sb = wpool.tile([P, DO, D], FP8, tag="wk")
                nc.sync.dma_start(wk_sb, wr["wkT"])
                wv_sb = wpool.tile([P, DO, D], FP8, tag="wv")
                nc.sync.dma_start(wv_sb, wr["wvT"])
                psq_pool = pha.enter_context(tc.tile_pool(name="psAq", bufs=3, space="PSUM"))
                ps = pha.enter_context(tc.tile_pool(name="psA", bufs=2, space="PSUM"))
                ksp = pha.enter_context(tc.tile_pool(name="ksA", bufs=1, space="PSUM"))
                ps_ks = ksp.tile([1, D], F32, tag="ksrow")

                def q_chunk(it, x_t, dc):
                    n0 = it * NT
                    ps_q = psq_pool.tile([P, NT], F32, tag="psq")
                    for kp in range(0, DO, 2):
                        nc.tensor.matmul(ps_q, wq_sb[:, kp:kp + 2, dc * P:(dc + 1) * P],
                                         x_t[:, kp:kp + 2, :], start=(kp == 0),
                                         stop=(kp == DO - 2 and not use_bq),
                                         perf_mode=DR)
                    if use_bq:
                        nc.tensor.matmul(ps_q, rows[0:1, dc * P:(dc + 1) * P],
                                         ones_row[0:1, :], start=False, stop=True)
                    m_t = ev.tile([P, NT], BF16, tag="mA")
                    nc.vector.tensor_scalar_min(m_t, ps_q, 0.0)
                    e_t = ev.tile([P, NT], BF16, tag="eA")
                    nc.scalar.activation(e_t, m_t, AF.Exp)
                    # elu(q)+1 = max(q,0) + exp(min(q,0)); spill chunk to HBM
                    qp_c = ev.tile([P, NT], FP8, tag="qpA")
                    nc.vector.scalar_tensor_tensor(qp_c, ps_q, 0.0, e_t,
                                                   OP.max, OP.add)
                    nc.sync.dma_start(qp_r[:, dc, n0:n0 + NT], qp_c)

                def kv_chunk(it, x_t, ch, half):
                    cg = it * NCH + ch
                    cs = slice(ch * P, (ch + 1) * P)
                    hs = slice(half * 512, (half + 1) * 512)
                    ps_k = ps.tile([P, 512], F32, tag="pskv")
                    for kp in range(0, DO, 2):
                        nc.tensor.matmul(ps_k, x_t[:, kp:kp + 2, cs],
                                         wk_sb[:, kp:kp + 2, hs],
                                         start=(kp == 0),
                                         stop=(kp == DO - 2 and not use_bk),
                                         perf_mode=DR)
                    if use_bk:
                        nc.tensor.matmul(ps_k, ones_1p_bf[0:1, :],
                                         rows[0:1, D + half * 512:D + (half + 1) * 512],
                                         start=False, stop=True)
                    m2 = ev.tile([P, 512], BF16, tag="mA")
                    nc.vector.tensor_scalar_min(m2, ps_k, 0.0)
                    e2 = ev.tile([P, 512], BF16, tag="eA")
                    nc.scalar.activation(e2, m2, AF.Exp)
                    nc.vector.scalar_tensor_tensor(kp_full[:, cg, hs],
                                                   ps_k, 0.0,
                                                   e2, OP.max, OP.add)

                    ps_v = ps.tile([P, 512], F32, tag="pskv")
                    for kp in range(0, DO, 2):
                        nc.tensor.matmul(ps_v, x_t[:, kp:kp + 2, cs],
                                         wv_sb[:, kp:kp + 2, hs],
                                         start=(kp == 0),
                                         stop=(kp == DO - 2 and not use_bv),
                                         perf_mode=DR)
                    if use_bv:
                        nc.tensor.matmul(ps_v, ones_1p_bf[0:1, :],
                                         rows[0:1, 2 * D + half * 512:2 * D + (half + 1) * 512],
                                         start=False, stop=True)
                    nc.scalar.activation(v_full[:, cg, hs], ps_v, AF.Copy)
                    # K_sum accumulation as a [1, D] row (tokens on partitions)
                    nc.tensor.matmul(ps_ks[0:1, hs], ones_col,
                                     kp_full[:, cg, hs],
                                     start=(cg == 0),
                                     stop=(cg == TOTCH - 1))

                for it in range(NTILES):
                    if it + 1 < NTILES:
                        load_x8(it + 1)
                    x_t = xts.pop(it)
                    for i in range(DO):
                        q_chunk(it, x_t, i)
                        kv_chunk(it, x_t, i // 2, i % 2)
                nc.scalar.activation(ksrow_sb, ps_ks[0:1, :], AF.Copy)
                # transpose K_sum row -> per-partition column layout [P, DO]
                ps_ksc = ksp.tile([P, DO], F32, tag="kscol")
                for dc in range(DO):
                    nc.tensor.matmul(ps_ksc[:, dc:dc + 1],
                                     ksrow_sb[0:1, dc * P:(dc + 1) * P],
                                     ones_one[0:1, 0:1], start=True, stop=True)
                nc.scalar.activation(ksum_sb[:, :, 0], ps_ksc, AF.Copy,
                                     scale=KSUM_SCALE)

            # ---- hoisted phase-C prefetches (overlap with B) ----
            nc.sync.dma_start(pp, pp_d[:])
            nc.sync.dma_start(diags, diags_d[:])
            nc.sync.dma_start(diagsb, diagsb_d[:])
            w1_sb = wpoolC.tile([P, DO, D], FP8, tag="w1")
            nc.sync.dma_start(w1_sb, wr["w1T"])
            w2_sb = wpoolC.tile([P, DO, D], BF16, tag="w2")
            nc.sync.dma_start(w2_sb, wr["w2T"])
            xc_tiles = {}

            def load_xc(it):
                n0 = it * NT
                x_t = ioC.tile([P, DO, W], BF16, tag="xC", name=f"x_{it}")
                x8_t = ioC8.tile([P, DO, W8], FP8, tag="x8C", name=f"x8c_{it}")
                lo, hi = n0 - 2, n0 + NT + 2
                if lo < 0:
                    nc.vector.memset(x_t[:, :, 0:2], 0.0)
                    nc.sync.dma_start(x_t[:, :, 2:W], xT[:, :, 0:hi])
                    nc.vector.memset(x8_t[:, :, 0:2], 0.0)
                    nc.sync.dma_start(x8_t[:, :, 2:W], x8r[:, :, 0:hi])
                elif hi > N:
                    nc.vector.memset(x_t[:, :, W - 2:W], 0.0)
                    nc.sync.dma_start(x_t[:, :, 0:W - 2], xT[:, :, lo:N])
                    nc.vector.memset(x8_t[:, :, W - 2:W], 0.0)
                    nc.sync.dma_start(x8_t[:, :, 0:W - 2], x8r[:, :, lo:N])
                else:
                    nc.sync.dma_start(x_t, xT[:, :, lo:hi])
                    nc.sync.dma_start(x8_t[:, :, 0:W], x8r[:, :, lo:hi])
                xc_tiles[it] = (x_t, x8_t)

            load_xc(0)

            # ------------ Phase B: KV accumulation (fp8 DR, SBUF src) ------------
            with ExitStack() as phb:
                psb = phb.enter_context(tc.tile_pool(name="psB", bufs=1, space="PSUM"))
                for eh in range(D // 512):
                    hs = slice(eh * 512, (eh + 1) * 512)
                    kv_ps = [psb.tile([P, 512], F32, tag=f"kvps{d}", name=f"kvps{d}_{eh}")
                             for d in range(DO)]
                    for chp in range(0, TOTCH, 2):
                        for dc in range(DO):
                            nc.tensor.matmul(kv_ps[dc],
                                             kp_full[:, chp:chp + 2, dc * P:(dc + 1) * P],
                                             v_full[:, chp:chp + 2, hs],
                                             start=(chp == 0),
                                             stop=(chp == TOTCH - 2), perf_mode=DR)
                    for dc in range(DO):
                        nc.scalar.activation(kv_sb[:, dc, hs], kv_ps[dc], AF.Copy,
                                             scale=KV_SCALE)

        # ---------------- Phase C: conv'' + local MLP + token mixer ----------------
        with phC as ph:
            pipe = ph.enter_context(tc.tile_pool(name="pipeC", bufs=2))
            mid = ph.enter_context(tc.tile_pool(name="midC", bufs=1))
            sm = ph.enter_context(tc.tile_pool(name="smC", bufs=1))
            ps = ph.enter_context(tc.tile_pool(name="psC", bufs=2, space="PSUM"))
            pst = ph.enter_context(tc.tile_pool(name="pstC", bufs=1, space="PSUM"))

            def c_front(it):
                n0 = it * NT
                x_t, x8_t = xc_tiles.pop(it)
                acc = acc_full[:, :, n0:n0 + NT]
                # diffusion dwconv'' center tap + bias on ACT (side taps join
                # the c_back PSUM group as diagonal matmuls)
                for o in range(DO):
                    nc.scalar.activation(acc[:, o, :], x_t[:, o, 2:NT + 2],
                                         AF.Identity, bias=pp[:, o, CB:CB + 1],
                                         scale=pp[:, o, C1i:C1i + 1])

                # local MLP first half (fp8 DoubleRow; w1 pre-scaled x16)
                h1_t = pipe.tile([P, DO, NT], BF16, tag="h1", name=f"h1_{it}")
                for dc in range(DO):
                    ps_h = ps.tile([P, NT], F32, tag="psh1", name=f"psh1_{it}_{dc}")
                    for kp in range(0, DO, 2):
                        nc.tensor.matmul(ps_h, w1_sb[:, kp:kp + 2, dc * P:(dc + 1) * P],
                                         x8_t[:, kp:kp + 2, 2:NT + 2],
                                         start=(kp == 0), stop=(kp == DO - 2),
                                         perf_mode=DR)
                    nc.scalar.activation(h1_t[:, dc, :], ps_h, AF.Gelu,
                                         bias=pp[:, dc, LUB1:LUB1 + 1],
                                         scale=1.0 / 16.0)

                # token mixer LN stats via fp8 DR on x8 (ones=1; /D at readout)
                sq_t = mid.tile([P, DO, W], FP8, tag="sq8", name=f"sq_{it}")
                nc.scalar.activation(sq_t, x8_t[:, :, 0:W], AF.Square)
                ps_m = pst.tile([P, W], F32, tag="psm", name=f"psm_{it}")
                stats_mm8(ps_m, x8_t[:, :, 0:W], W)
                ps_s = pst.tile([P, W], F32, tag="pss", name=f"pss_{it}")
                stats_mm8(ps_s, sq_t, W)
                m_sb = sm.tile([P, W], BF16, tag="msb", name=f"msb_{it}")
                nc.scalar.activation(m_sb, ps_m, AF.Copy, scale=1.0 / D)
                var = sm.tile([P, W], F32, tag="var", name=f"var_{it}")
                nc.scalar.activation(var, ps_m, AF.Square, scale=1.0 / D)
                nc.vector.scalar_tensor_tensor(var, ps_s, 1.0 / D, var,
                                               OP.mult, OP.subtract)
                nc.scalar.activation(var, var, AF.Sqrt, bias=eps_ln[:, 0:1])
                nc.vector.reciprocal_approx_fast(out=var, in_=var)
                rstd = sm.tile([P, W], BF16, tag="rstd", name=f"rstd_{it}")
                nc.vector.tensor_copy(rstd, var)
                # xm = (x - m) * rstd  (tm gamma folded into conv1 taps)
                xm_t = mid.tile([P, DO, W], BF16, tag="tokC", name=f"xm_{it}")
                nc.vector.tensor_sub(xm_t, x_t,
                                     m_sb[:, None, :].broadcast_to([P, DO, W]))
                nc.vector.tensor_mul(xm_t, xm_t,
                                     rstd[:, None, :].broadcast_to([P, DO, W]))
                if use_tmb:
                    for o in range(DO):
                        nc.vector.tensor_scalar_add(xm_t[:, o, :], xm_t[:, o, :],
                                                    pp[:, o, TMB:TMB + 1])
                # conv1: t_s[k] = conv1(xm)[k+1], k in [0, W-2)
                t_t = mid.tile([P, DO, W - 2], BF16, tag="tokD", name=f"t_{it}")
                for o in range(DO):
                    nc.scalar.activation(t_t[:, o, :], xm_t[:, o, 1:W - 1],
                                         AF.Identity, bias=pp[:, o, TCB1:TCB1 + 1],
                                         scale=pp[:, o, T1:T1 + 1])
                for o in range(DO):
                    nc.vector.scalar_tensor_tensor(t_t[:, o, :], xm_t[:, o, 0:W - 2],
                                                   pp[:, o, T0:T0 + 1],
                                                   t_t[:, o, :], OP.mult, OP.add)
                for o in range(DO):
                    nc.vector.scalar_tensor_tensor(t_t[:, o, :], xm_t[:, o, 2:W],
                                                   pp[:, o, T2:T2 + 1],
                                                   t_t[:, o, :], OP.mult, OP.add)
                t2_t = pipe.tile([P, DO, W - 2], FP8, tag="t2", name=f"t2_{it}")
                nc.scalar.activation(t2_t, t_t, AF.Gelu)
                if it == 0:
                    nc.vector.memset(t2_t[:, :, 0:1], 0.0)
                if it == NTILES - 1:
                    nc.vector.memset(t2_t[:, :, W - 3:W - 2], 0.0)
                return x_t, h1_t, t2_t

            def c_back(it, tiles):
                n0 = it * NT
                x_t, h1_t, t2_t = tiles
                for dc in range(DO):
                    ps_h = ps.tile([P, NT], F32, tag="psh2", name=f"psh2_{it}_{dc}")
                    for kc in range(DO):
                        nc.tensor.matmul(ps_h, w2_sb[:, kc, dc * P:(dc + 1) * P],
                                         h1_t[:, kc, :],
                                         start=(kc == 0), stop=False)
                    for tap in range(3):
                        nc.tensor.matmul(ps_h, diags[:, tap, dc, :],
                                         t2_t[:, dc, tap:NT + tap],
                                         start=False, stop=False)
                    # diffusion conv side taps as diagonal matmuls
                    nc.tensor.matmul(ps_h, diagsb[:, 0, dc, :],
                                     x_t[:, dc, 1:NT + 1], start=False, stop=False)
                    nc.tensor.matmul(ps_h, diagsb[:, 1, dc, :],
                                     x_t[:, dc, 3:NT + 3], start=False, stop=True)
                    nc.vector.tensor_add(acc_full[:, dc, n0:n0 + NT],
                                         acc_full[:, dc, n0:n0 + NT], ps_h)

            pend = {0: c_front(0)}
            for it in range(NTILES):
                if it + 1 < NTILES:
                    load_xc(it + 1)
                    pend[it + 1] = c_front(it + 1)
                c_back(it, pend.pop(it))

        # ---------------- Phase D: attention + LN1 + FFN + LN2 ----------------
        with ExitStack() as ph:
            wpoolD = ph.enter_context(tc.tile_pool(name="wD", bufs=1))
            f1_sb = wpoolD.tile([P, DO, D], BF16, tag="f1")
            nc.sync.dma_start(f1_sb, wr["f1T"])
            f2_sb = wpoolD.tile([P, DO, D], BF16, tag="f2")
            nc.sync.dma_start(f2_sb, wr["f2T"])
            ioD = ph.enter_context(tc.tile_pool(name="ioD", bufs=2))
            mid = ph.enter_context(tc.tile_pool(name="midD", bufs=1))
            sm = ph.enter_context(tc.tile_pool(name="smD", bufs=2))
            ps = ph.enter_context(tc.tile_pool(name="psD", bufs=2, space="PSUM"))
            psf_pool = ph.enter_context(tc.tile_pool(name="psfD", bufs=3, space="PSUM"))
            pst = ph.enter_context(tc.tile_pool(name="pstD", bufs=1, space="PSUM"))

            def d_front_a(it):
                """norm row, C1/norm fold, numerator halves 0-3."""
                n0 = it * NT
                qp_t = ioD.tile([P, DO, NT], FP8, tag="qpD", name=f"qp_{it}")
                nc.sync.dma_start(qp_t, qp_r[:, :, n0:n0 + NT])
                acc_t = acc_full[:, :, n0:n0 + NT]
                ps_n = pst.tile([P, NT], F32, tag="psrep", name=f"psn_{it}")
                for kc in range(DO):
                    nc.tensor.matmul(ps_n[0:1, :], ksum_sb[:, kc, :],
                                     qp_t[:, kc, :],
                                     start=(kc == 0), stop=(kc == DO - 1))
                nr = sm.tile([1, NT], F32, tag="nrD", name=f"nr_{it}")
                nc.vector.tensor_scalar_add(nr, ps_n[0:1, :], 1e-6 * KSUM_SCALE)
                rr = sm.tile([1, NT], F32, tag="rrD", name=f"rr_{it}")
                nc.vector.reciprocal_approx_fast(out=rr, in_=nr)
                rrb = sm.tile([1, NT], BF16, tag="rrbD", name=f"rrb_{it}")
                nc.vector.tensor_copy(rrb, rr)
                # rep = C1*KSUM_SCALE * (1/(norm*KSUM_SCALE)) = C1/norm
                ps_rep = pst.tile([P, NT], F32, tag="psrep", name=f"psrep_{it}")
                nc.tensor.matmul(ps_rep, repc_row[0:1, :], rrb, start=True,
                                 stop=True)
                rep_sb = mid.tile([P, NT], BF16, tag="repsb", name=f"rep_{it}")
                nc.scalar.activation(rep_sb, ps_rep, AF.Copy)
                nc.vector.tensor_mul(qp_t, qp_t,
                                     rep_sb[:, None, :].broadcast_to([P, DO, NT]))
                for ec in range(DO // 2):
                    ps_u = ps.tile([P, NT], F32, tag="psnum", name=f"psnum_{it}_{ec}")
                    for kp in range(0, DO, 2):
                        nc.tensor.matmul(ps_u, kv_sb[:, kp:kp + 2, ec * P:(ec + 1) * P],
                                         qp_t[:, kp:kp + 2, :],
                                         start=(kp == 0), stop=(kp == DO - 2),
                                         perf_mode=DR)
                    nc.vector.scalar_tensor_tensor(acc_t[:, ec, :], ps_u,
                                                   1.0 / (KV_SCALE * C1),
                                                   acc_t[:, ec, :], OP.mult, OP.add)
                return qp_t, acc_t

            def d_front_b(it, T):
                qp_t, acc_t = T
                for ec in range(DO // 2, DO):
                    ps_u = ps.tile([P, NT], F32, tag="psnum", name=f"psnum_{it}_{ec}")
                    for kp in range(0, DO, 2):
                        nc.tensor.matmul(ps_u, kv_sb[:, kp:kp + 2, ec * P:(ec + 1) * P],
                                         qp_t[:, kp:kp + 2, :],
                                         start=(kp == 0), stop=(kp == DO - 2),
                                         perf_mode=DR)
                    nc.vector.scalar_tensor_tensor(acc_t[:, ec, :], ps_u,
                                                   1.0 / (KV_SCALE * C1),
                                                   acc_t[:, ec, :], OP.mult, OP.add)
                return acc_t

            def d_mid(it, acc_t):
                """LN1 stats + apply -> y1 (bf16; n1 gamma folded into f1)."""
                acc_bf = mid.tile([P, DO, NT], BF16, tag="accbf", name=f"accbf_{it}")
                nc.scalar.activation(acc_bf, acc_t, AF.Copy)
                sq_t = mid.tile([P, DO, NT], FP8, tag="sqD8", name=f"sqD_{it}")
                nc.scalar.activation(sq_t, acc_t, AF.Square)
                ps_m1 = pst.tile([P, NT], F32, tag="psm1", name=f"psm1_{it}")
                stats_mm(ps_m1, onesD_bf, acc_bf, NT)
                ps_s1 = pst.tile([P, NT], F32, tag="pss1", name=f"pss1_{it}")
                stats_mm8(ps_s1, sq_t, NT)
                m1_sb = sm.tile([P, NT], BF16, tag="m1sb", name=f"m1_{it}")
                nc.scalar.activation(m1_sb, ps_m1, AF.Copy)
                var1 = sm.tile([P, NT], F32, tag="varD", name=f"var1_{it}")
                nc.scalar.activation(var1, ps_m1, AF.Square)
                nc.vector.scalar_tensor_tensor(var1, ps_s1, 1.0 / D, var1,
                                               OP.mult, OP.subtract)
                nc.scalar.activation(var1, var1, AF.Sqrt, bias=eps_ln[:, 0:1])
                nc.vector.reciprocal_approx_fast(out=var1, in_=var1)
                rstd1 = sm.tile([P, NT], BF16, tag="rstdb", name=f"rstdb_{it}")
                nc.vector.tensor_copy(rstd1, var1)
                y1_t = mid.tile([P, DO, NT], BF16, tag="y1", name=f"y1_{it}")
                nc.vector.tensor_sub(y1_t, acc_bf,
                                     m1_sb[:, None, :].broadcast_to([P, DO, NT]))
                nc.vector.tensor_mul(y1_t, y1_t,
                                     rstd1[:, None, :].broadcast_to([P, DO, NT]))
                if use_n1b:
                    for o in range(DO):
                        nc.vector.tensor_scalar_add(y1_t[:, o, :], y1_t[:, o, :],
                                                    pp[:, o, N1B:N1B + 1])
                return y1_t

            def d_ffn(it, y1_t):
                f1h_t = mid.tile([P, DO, NT], BF16, tag="f1h", name=f"f1h_{it}")
                for dc in range(DO):
                    ps_f = psf_pool.tile([P, NT], F32, tag="psf",
                                         name=f"psf1_{it}_{dc}")
                    for kc in range(DO):
                        nc.tensor.matmul(ps_f, f1_sb[:, kc, dc * P:(dc + 1) * P],
                                         y1_t[:, kc, :],
                                         start=(kc == 0), stop=(kc == DO - 1))
                    nc.scalar.activation(f1h_t[:, dc, :], ps_f, AF.Gelu,
                                         bias=pp[:, dc, FFB1:FFB1 + 1])
                y2_t = mid.tile([P, DO, NT], BF16, tag="y2", name=f"y2_{it}")
                for dc in range(DO):
                    ps_f = psf_pool.tile([P, NT], F32, tag="psf",
                                         name=f"psf2_{it}_{dc}")
                    for kc in range(DO):
                        nc.tensor.matmul(ps_f, f2_sb[:, kc, dc * P:(dc + 1) * P],
                                         f1h_t[:, kc, :],
                                         start=(kc == 0), stop=(kc == DO - 1))
                    if use_n1g:
                        # y2 = y1*g + f2(h); (residual gamma must be re-applied)
                        nc.vector.scalar_tensor_tensor(y2_t[:, dc, :],
                                                       y1_t[:, dc, :],
                                                       pp[:, dc, N1G:N1G + 1],
                                                       ps_f, OP.mult, OP.add)
                    else:
                        nc.vector.scalar_tensor_tensor(y2_t[:, dc, :], ps_f,
                                                       pp[:, dc, FFB2:FFB2 + 1],
                                                       y1_t[:, dc, :], OP.add, OP.add)
                return y2_t

            def d_back(it, y2_t, c0=0, cw=None):
                if cw is None:
                    cw = NT
                n0 = it * NT + c0
                y2s = y2_t[:, :, c0:c0 + cw]
                sq2_t = mid.tile([P, DO, NT], FP8, tag="sq28", name=f"sq2_{it}_{c0}")
                nc.scalar.activation(sq2_t[:, :, 0:cw], y2s, AF.Square)
                ps_m2 = pst.tile([P, NT], F32, tag="psm1", name=f"psm2_{it}_{c0}")
                stats_mm(ps_m2[:, 0:cw], onesD_bf, y2s, cw)
                ps_s2 = pst.tile([P, NT], F32, tag="pss1", name=f"pss2_{it}_{c0}")
                stats_mm8(ps_s2[:, 0:cw], sq2_t[:, :, 0:cw], cw)
                m2_sb = sm.tile([P, NT], BF16, tag="m2sb", name=f"m2_{it}_{c0}")
                nc.scalar.activation(m2_sb[:, 0:cw], ps_m2[:, 0:cw], AF.Copy)
                var2 = sm.tile([P, NT], F32, tag="varD", name=f"var2_{it}_{c0}")
                nc.scalar.activation(var2[:, 0:cw], ps_m2[:, 0:cw], AF.Square)
                nc.vector.scalar_tensor_tensor(var2[:, 0:cw], ps_s2[:, 0:cw],
                                               1.0 / D, var2[:, 0:cw],
                                               OP.mult, OP.subtract)
                nc.scalar.activation(var2[:, 0:cw], var2[:, 0:cw], AF.Sqrt,
                                     bias=eps_ln[:, 0:1])
                nc.vector.reciprocal_approx_fast(out=var2[:, 0:cw],
                                                 in_=var2[:, 0:cw])
                rstd2 = sm.tile([P, NT], BF16, tag="rstd2", name=f"rstd2_{it}_{c0}")
                nc.vector.tensor_copy(rstd2[:, 0:cw], var2[:, 0:cw])
                yo_t = mid.tile([P, DO, NT], BF16, tag="yo", name=f"yo_{it}_{c0}")
                yo = yo_t[:, :, 0:cw]
                nc.vector.tensor_sub(yo, y2s,
                                     m2_sb[:, None, 0:cw].broadcast_to([P, DO, cw]))
                if use_n2g:
                    for o in range(DO):
                        nc.vector.scalar_tensor_tensor(yo[:, o, :], yo[:, o, :],
                                                       pp[:, o, N2G:N2G + 1],
                                                       rstd2[:, 0:cw],
                                                       OP.mult, OP.mult)
                else:
                    nc.vector.tensor_mul(yo, yo,
                                         rstd2[:, None, 0:cw].broadcast_to([P, DO, cw]))
                if use_n2b:
                    for o in range(DO):
                        nc.vector.tensor_scalar_add(yo[:, o, :], yo[:, o, :],
                                                    pp[:, o, N2B:N2B + 1])
                nc.sync.dma_start(yT[:, :, n0:n0 + cw], yo)

            # Pipeline: tile t+1's numerator halves run under tile t's LN/FFN,
            # and tile t+1's LN1 chain is issued before tile t's LN2 so the
            # FFN matmuls of t+1 can flow while t's LN2 drains on DVE/ACT.
            acc_cur = d_front_b(0, d_front_a(0))
            y1_cur = d_mid(0, acc_cur)
            for it in range(NTILES):
                Tnext = d_front_a(it + 1) if it + 1 < NTILES else None
                y2_cur = d_ffn(it, y1_cur)
                if Tnext is not None:
                    acc_next = d_front_b(it + 1, Tnext)
                    y1_cur = d_mid(it + 1, acc_next)
                    d_back(it, y2_cur)
                else:
                    # last tile: split LN2 so the final chain+DMA pipelines
                    d_back(it, y2_cur, 0, NT // 2)
                    d_back(it, y2_cur, NT // 2, NT // 2)

    nc.compile()
    return nc


def make_in_maps(inputs, n_cores=8):
    """Host-side preprocessing: fold constants, transpose, cast, shard."""
    x = np.asarray(inputs["x"], np.float32)
    B, N, D_ = x.shape
    dt = float(np.asarray(inputs["delta_t"]))

    def g(k):
        return np.asarray(inputs[k], np.float32)

    diff_w, diff_b = g("diff_w"), g("diff_b")
    tm_w1, tm_cb1 = g("tm_w1"), g("tm_cb1")
    tm_w2, tm_cb2 = g("tm_w2"), g("tm_cb2")
    tm_g = g("tm_g")
    n1_g, n2_g = g("n1_g"), g("n2_g")

    pp = np.zeros((P, DO, NPARAM), np.float32)

    def put(i, v):
        pp[:, :, i] = v.reshape(DO, P).T

    put(C0, dt * diff_w[:, 0, 0])
    put(C1i, dt * diff_w[:, 0, 1] + (1.0 - dt))
    put(C2, dt * diff_w[:, 0, 2])
    put(CB, dt * diff_b + g("lu_b2") + tm_cb2)
    # token-mixer gamma folded into the conv1 taps
    put(T0, tm_w1[:, 0, 0] * tm_g)
    put(T1, tm_w1[:, 0, 1] * tm_g)
    put(T2, tm_w1[:, 0, 2] * tm_g)
    put(TCB1, tm_cb1)
    put(U0, tm_w2[:, 0, 0])
    put(U1, tm_w2[:, 0, 1])
    put(U2, tm_w2[:, 0, 2])
    put(TMG, tm_g)
    put(TMB, g("tm_beta"))
    put(N1G, n1_g)
    put(N1B, g("n1_b"))
    put(N2G, n2_g)
    put(N2B, g("n2_b"))
    put(LUB1, g("lu_b1"))
    put(FFB1, g("ff_b1"))
    put(FFB2, g("ff_b2"))

    # diags (fp8): token-mixer conv2 taps; diagsb (bf16): diffusion side taps
    diags = np.zeros((P, 3, DO, P), np.float32)
    diagsb = np.zeros((P, 2, DO, P), np.float32)
    idx = np.arange(P)
    for tap in range(3):
        for dc in range(DO):
            diags[idx, tap, dc, idx] = tm_w2[dc * P + idx, 0, tap]
    for dc in range(DO):
        diagsb[idx, 0, dc, idx] = dt * diff_w[dc * P + idx, 0, 0]
        diagsb[idx, 1, dc, idx] = dt * diff_w[dc * P + idx, 0, 2]
    diags = np.clip(diags, -240, 240).astype(FP8_NP)
    diagsb = diagsb.astype(BF16_NP)

    rows = np.zeros((1, 3 * D), np.float32)
    rows[0, 0:D] = g("bq")
    rows[0, D:2 * D] = g("bk")
    rows[0, 2 * D:3 * D] = g("bv")
    rows = rows.astype(BF16_NP)

    use_n1g = bool(np.any(n1_g != 1.0))
    use_n2g = bool(np.any(n2_g != 1.0))

    wt = {}
    for name, key in (("w2T", "lu_w2"), ("f2T", "ff_w2")):
        wt[name] = np.ascontiguousarray(g(key).T).astype(BF16_NP)
    # n1 gamma folded into ff_w1 input rows (when gamma != 1)
    f1 = g("ff_w1")
    if use_n1g:
        f1 = f1 * n1_g[None, :]
    wt["f1T"] = np.ascontiguousarray(f1.T).astype(BF16_NP)
    # w1 shipped as fp8 pre-scaled x16 (consumer applies 1/16 via ACT scale)
    wt["w1T"] = np.ascontiguousarray(
        np.clip(g("lu_w1").T * 16.0, -240, 240)).astype(FP8_NP)
    for name, key in (("wqT", "wq"), ("wkT", "wk"), ("wvT", "wv")):
        wt[name] = np.ascontiguousarray(
            np.clip(g(key).T, -240, 240)).astype(FP8_NP)

    xT = np.ascontiguousarray(x.transpose(0, 2, 1)).astype(BF16_NP)
    x8 = np.clip(xT.astype(np.float32), -240, 240).astype(FP8_NP)

    flags = dict(
        use_bq=bool(np.any(g("bq"))),
        use_bk=bool(np.any(g("bk"))),
        use_bv=bool(np.any(g("bv"))),
        use_tmb=bool(np.any(g("tm_beta"))),
        use_n1b=bool(np.any(g("n1_b"))),
        use_n2b=bool(np.any(g("n2_b"))),
        use_n1g=use_n1g,
        use_n2g=use_n2g,
    )

    shared = {**wt, "pp": pp, "rows": rows, "diags": diags,
              "diagsb": diagsb}
    in_maps = [{**shared, "x_T": xT[b], "x8": x8[b]} for b in range(n_cores)]
    return in_maps, flags, (B, N)


_NC_CACHE = {}


def kernel(**inputs):
    in_maps, flags, (B, N) = make_in_maps(inputs)
    key = (N, tuple(sorted(flags.items())))
    if key not in _NC_CACHE:
        _NC_CACHE[key] = build_nc(N=N, NT=512, **flags)
    nc = _NC_CACHE[key]
    res = run_bass_kernel_spmd(nc, in_maps, list(range(B)))
    y = np.stack([res.results[b]["y_T"] for b in range(B)])
    return np.ascontiguousarray(y.transpose(0, 2, 1)).astype(np.float32)


# revision 55
# speedup vs baseline: 1.0617x; 1.0244x over previous
"""Trainium2 Bass kernel for nn_DiffuRNNLayer (B=8, N=2048, D=1024).

Sharding: data-parallel over batch — one batch element per NeuronCore (8 cores).
Per-core kernel works in "layout B" ([d on partitions, n on free]) with the
input pre-transposed on the host.  Phases per core (all intermediates
SBUF-resident, no HBM spills):
  A: Q/K/V projections in fp8 DoubleRow (+elu+1) -> qp/kp/v fp8 SBUF tiles;
     K_sum accumulation.
  B: KV = Kp^T V accumulation straight from SBUF (fp8 DoubleRow, 2 e-halves);
     kv_sb kept as fp8 with a 1/4 scale.
  C: acc = dwconv''(x) + MLP(x) + tokenmixer(LN(x)) into a resident f32 acc
     (diffusion residual and constant per-channel biases folded into the
     dwconv'' taps host-side; tokenmixer gamma folded into the conv1 taps).
  D: attn numerator (fp8 DoubleRow) with C1/norm folded into Qp, acc += attn;
     LN1 (gamma folded into ff_w1); FFN residual; LN2; write y^T (bf16).
Host transposes x/weights in, and the output back out.
"""

import numpy as np
import ml_dtypes
from contextlib import ExitStack

import concourse.bass as bass
import concourse.bacc as bacc
import concourse.tile as tile
import concourse.mybir as mybir
from concourse.bass_utils import run_bass_kernel_spmd

F32 = mybir.dt.float32
BF16 = mybir.dt.bfloat16
FP8 = mybir.dt.float8e4
AF = mybir.ActivationFunctionType
OP = mybir.AluOpType
DR = mybir.MatmulPerfMode.DoubleRow
BF16_NP = ml_dtypes.bfloat16
FP8_NP = ml_dtypes.float8_e4m3

P = 128
D = 1024
DO = D // P  # 8 chunks of the channel dim

KV_SCALE = 0.25          # kv_sb stored as KV/4 in fp8
KSUM_SCALE = 1.0 / 16.0  # ksum stored as ksum/16 in fp8
C1 = float(2.0 ** 21)    # qp fold: qp * C1/norm stays O(1) for fp8

# pp param-plane indices (per-partition params, laid out [128, DO, NP])
(C0, C1i, C2, CB, T0, T1, T2, TCB1, U0, U1, U2,
 TMG, TMB, N1G, N1B, N2G, N2B, LUB1, FFB1, FFB2) = range(20)
NPARAM = 20


def build_nc(N=2048, NT=512, use_bq=False, use_bk=False, use_bv=False,
             use_tmb=False, use_n1b=False, use_n2b=False,
             use_n1g=False, use_n2g=False, debug=False):
    NTILES = N // NT
    NCH = NT // P          # 128-token chunks per tile
    TOTCH = N // P
    W = NT + 4             # phase-C tile width with +-2 halo
    W8 = NT + 8            # x8 tile width padded so dual-fp8 LDW strides are 8-aligned
    assert N % NT == 0 and NT % P == 0

    nc = bacc.Bacc(None, target_bir_lowering=False, debug=debug)

    xT_d = nc.dram_tensor("x_T", [D, N], BF16, kind="ExternalInput")
    x8_d = nc.dram_tensor("x8", [D, N], FP8, kind="ExternalInput")
    w_d = {}
    for name in ("wqT", "wkT", "wvT", "w1T"):
        w_d[name] = nc.dram_tensor(name, [D, D], FP8, kind="ExternalInput")
    for name in ("w2T", "f1T", "f2T"):
        w_d[name] = nc.dram_tensor(name, [D, D], BF16, kind="ExternalInput")
    pp_d = nc.dram_tensor("pp", [P, DO, NPARAM], F32, kind="ExternalInput")
    diags_d = nc.dram_tensor("diags", [P, 3, DO, P], FP8, kind="ExternalInput")
    diagsb_d = nc.dram_tensor("diagsb", [P, 2, DO, P], BF16, kind="ExternalInput")
    rows_d = nc.dram_tensor("rows", [1, 3 * D], BF16, kind="ExternalInput")
    yT_d = nc.dram_tensor("y_T", [D, N], BF16, kind="ExternalOutput")
    qp_sp = nc.dram_tensor("qp_sp", [D, N], FP8)
    qp_r = qp_sp.rearrange("(o p) n -> p o n", p=P)

    xT = xT_d.rearrange("(o p) n -> p o n", p=P)
    x8r = x8_d.rearrange("(o p) n -> p o n", p=P)
    wr = {k: v.rearrange("(o p) n -> p o n", p=P) for k, v in w_d.items()}
    yT = yT_d.rearrange("(o p) n -> p o n", p=P)

    with tile.TileContext(nc) as tc, ExitStack() as top:
        persist = top.enter_context(tc.tile_pool(name="persist", bufs=1))
        pp = persist.tile([P, DO, NPARAM], F32)
        rows = ones_row = ones_1p_bf = None
        if use_bq or use_bk or use_bv:
            rows = persist.tile([1, 3 * D], BF16)
            nc.sync.dma_start(rows, rows_d[:])
            ones_row = persist.tile([1, NT], BF16)
            nc.vector.memset(ones_row, 1.0)
            ones_1p_bf = persist.tile([1, P], BF16)
            nc.vector.memset(ones_1p_bf, 1.0)
        repc_row = persist.tile([1, P], BF16)
        nc.vector.memset(repc_row, C1 * KSUM_SCALE)
        ones_col = persist.tile([P, 1], FP8)
        nc.vector.memset(ones_col, 1.0)
        ones8 = persist.tile([P, 2, P], FP8)
        nc.vector.memset(ones8, 1.0)
        ones_one = persist.tile([1, 1], BF16)
        nc.vector.memset(ones_one, 1.0)
        ksrow_sb = persist.tile([1, D], BF16)
        onesD_bf = persist.tile([P, P], BF16)
        nc.vector.memset(onesD_bf, 1.0 / D)
        eps_ln = persist.tile([P, 1], F32)
        nc.vector.memset(eps_ln, 1e-5)
        kv_sb = persist.tile([P, DO, D], FP8)
        ksum_sb = persist.tile([P, DO, 1], FP8)
        diags = persist.tile([P, 3, DO, P], FP8)
        diagsb = persist.tile([P, 2, DO, P], BF16)

        def stats_mm(psum, lhs_ones, rhs3, width):
            """Accumulate over DO k-chunks: psum[:, j] = sum over channel dim
            times lhs value, replicated across partitions.  rhs3: [P, DO, w]."""
            for c0 in range(0, width, 512):
                cw = min(512, width - c0)
                for kc in range(DO):
                    nc.tensor.matmul(psum[:, c0:c0 + cw], lhs_ones,
                                     rhs3[:, kc, c0:c0 + cw],
                                     start=(kc == 0), stop=(kc == DO - 1))

        def stats_mm8(psum, rhs3, width):
            """fp8 DoubleRow stats: psum[:, j] = SUM over channels (ones=1.0;
            divide by D at consumption).  rhs3: [P, DO, width] fp8."""
            for c0 in range(0, width, 512):
                cw = min(512, width - c0)
                for kp in range(0, DO, 2):
                    nc.tensor.matmul(psum[:, c0:c0 + cw], ones8[:, :, 0:P],
                                     rhs3[:, kp:kp + 2, c0:c0 + cw],
                                     start=(kp == 0), stop=(kp == DO - 2),
                                     perf_mode=DR)

        # Phase-C pools and the resident acc created up-front (LIFO pool
        # order: they outlive kvres); their DMAs are issued between A and B.
        cd = top.enter_context(ExitStack())
        accpool = cd.enter_context(tc.tile_pool(name="accres", bufs=1))
        acc_full = accpool.tile([P, DO, N], F32)
        phC = ExitStack()
        wpoolC = phC.enter_context(tc.tile_pool(name="wC", bufs=1))
        ioC = phC.enter_context(tc.tile_pool(name="ioC", bufs=3))
        ioC8 = phC.enter_context(tc.tile_pool(name="ioC8", bufs=2))

        # ============ Phases A+B (kp/v SBUF-resident) ============
        with ExitStack() as ph:
            kvres = ph.enter_context(tc.tile_pool(name="kvres", bufs=1))
            kp_full = kvres.tile([P, TOTCH, D], FP8)
            v_full = kvres.tile([P, TOTCH, D], FP8)

            # ---------------- Phase A: QKV (fp8 DoubleRow) ----------------
            # Q-chunks and K/V-chunks are interleaved so the DVE-paced elu
            # chains of one overlap the PE-paced matmuls of the other.
            with ExitStack() as pha:
                wpool = pha.enter_context(tc.tile_pool(name="wA", bufs=1))
                io = pha.enter_context(tc.tile_pool(name="ioA", bufs=2))
                ev = pha.enter_context(tc.tile_pool(name="evA", bufs=2))
                wq_sb = wpool.tile([P, DO, D], FP8, tag="wq")
                nc.sync.dma_start(wq_sb, wr["wqT"])
                xts = {}

                def load_x8(it):
                    t = io.tile([P, DO, NT], FP8, tag="xA", name=f"x8a_{it}")
                    nc.sync.dma_start(t, x8r[:, :, it * NT:(it + 1) * NT])
                    xts[it] = t

                load_x8(0)
                wk_sb = wpool.tile([P, DO, D], FP8, tag="wk")
                nc.sync.dma_start(wk_sb, wr["wkT"])
                wv_sb = wpool.tile([P, DO, D], FP8, tag="wv")
                nc.sync.dma_start(wv_sb, wr["wvT"])
                psq_pool = pha.enter_context(tc.tile_pool(name="psAq", bufs=3, space="PSUM"))
                ps = pha.enter_context(tc.tile_pool(name="psA", bufs=2, space="PSUM"))
                ksp = pha.enter_context(tc.tile_pool(name="ksA", bufs=1, space="PSUM"))
                ps_ks = ksp.tile([1, D], F32, tag="ksrow")

                def q_chunk(it, x_t, dc):
                    n0 = it * NT
                    ps_q = psq_pool.tile([P, NT], F32, tag="psq")
                    for kp in range(0, DO, 2):
                        nc.tensor.matmul(ps_q, wq_sb[:, kp:kp + 2, dc * P:(dc + 1) * P],
                                         x_t[:, kp:kp + 2, :], start=(kp == 0),
                                         stop=(kp == DO - 2 and not use_bq),
                                         perf_mode=DR)
                    if use_bq:
                        nc.tensor.matmul(ps_q, rows[0:1, dc * P:(dc + 1) * P],
                                         ones_row[0:1, :], start=False, stop=True)
                    m_t = ev.tile([P, NT], BF16, tag="mA")
                    nc.vector.tensor_scalar_min(m_t, ps_q, 0.0)
                    e_t = ev.tile([P, NT], BF16, tag="eA")
                    nc.scalar.activation(e_t, m_t, AF.Exp)
                    # elu(q)+1 = max(q,0) + exp(min(q,0)); spill chunk to HBM
                    qp_c = ev.tile([P, NT], FP8, tag="qpA")
                    nc.vector.scalar_tensor_tensor(qp_c, ps_q, 0.0, e_t,
                                                   OP.max, OP.add)
                    nc.sync.dma_start(qp_r[:, dc, n0:n0 + NT], qp_c)

                def kv_chunk(it, x_t, ch, half):
                    cg = it * NCH + ch
                    cs = slice(ch * P, (ch + 1) * P)
                    hs = slice(half * 512, (half + 1) * 512)
                    ps_k = ps.tile([P, 512], F32, tag="pskv")
                    for kp in range(0, DO, 2):
                        nc.tensor.matmul(ps_k, x_t[:, kp:kp + 2, cs],
                                         wk_sb[:, kp:kp + 2, hs],
                                         start=(kp == 0),
                                         stop=(kp == DO - 2 and not use_bk),
                                         perf_mode=DR)
                    if use_bk:
                        nc.tensor.matmul(ps_k, ones_1p_bf[0:1, :],
                                         rows[0:1, D + half * 512:D + (half + 1) * 512],
                                         start=False, stop=True)
                    m2 = ev.tile([P, 512], BF16, tag="mA")
                    nc.vector.tensor_scalar_min(m2, ps_k, 0.0)
                    e2 = ev.tile([P, 512], BF16, tag="eA")
                    nc.scalar.activation(e2, m2, AF.Exp)
                    nc.vector.scalar_tensor_tensor(kp_full[:, cg, hs],
                                                   ps_k, 0.0,
                                                   e2, OP.max, OP.add)

                    ps_v = ps.tile([P, 512], F32, tag="pskv")
                    for kp in range(0, DO, 2):
                        nc.tensor.matmul(ps_v, x_t[:, kp:kp + 2, cs],
                                         wv_sb[:, kp:kp + 2, hs],
                                         start=(kp == 0),
                                         stop=(kp == DO - 2 and not use_bv),
                                         perf_mode=DR)
                    if use_bv:
                        nc.tensor.matmul(ps_v, ones_1p_bf[0:1, :],
                                         rows[0:1, 2 * D + half * 512:2 * D + (half + 1) * 512],
                                         start=False, stop=True)
                    nc.scalar.activation(v_full[:, cg, hs], ps_v, AF.Copy)
                    # K_sum accumulation as a [1, D] row (tokens on partitions)
                    nc.tensor.matmul(ps_ks[0:1, hs], ones_col,
                                     kp_full[:, cg, hs],
                                     start=(cg == 0),
                                     stop=(cg == TOTCH - 1))

                for it in range(NTILES):
                    if it + 1 < NTILES:
                        load_x8(it + 1)
                    x_t = xts.pop(it)
                    for i in range(DO):
                        q_chunk(it, x_t, i)
                        kv_chunk(it, x_t, i // 2, i % 2)
                nc.scalar.activation(ksrow_sb, ps_ks[0:1, :], AF.Copy)
                # transpose K_sum row -> per-partition column layout [P, DO]
                ps_ksc = ksp.tile([P, DO], F32, tag="kscol")
                for dc in range(DO):
                    nc.tensor.matmul(ps_ksc[:, dc:dc + 1],
                                     ksrow_sb[0:1, dc * P:(dc + 1) * P],
                                     ones_one[0:1, 0:1], start=True, stop=True)
                nc.scalar.activation(ksum_sb[:, :, 0], ps_ksc, AF.Copy,
                                     scale=KSUM_SCALE)

            # ---- hoisted phase-C prefetches (overlap with B) ----
            nc.sync.dma_start(pp, pp_d[:])
            nc.sync.dma_start(diags, diags_d[:])
            nc.sync.dma_start(diagsb, diagsb_d[:])
            w1_sb = wpoolC.tile([P, DO, D], FP8, tag="w1")
            nc.sync.dma_start(w1_sb, wr["w1T"])
            w2_sb = wpoolC.tile([P, DO, D], BF16, tag="w2")
            nc.sync.dma_start(w2_sb, wr["w2T"])
            xc_tiles = {}

            def load_xc(it):
                n0 = it * NT
                x_t = ioC.tile([P, DO, W], BF16, tag="xC", name=f"x_{it}")
                x8_t = ioC8.tile([P, DO, W8], FP8, tag="x8C", name=f"x8c_{it}")
                lo, hi = n0 - 2, n0 + NT + 2
                if lo < 0:
                    nc.vector.memset(x_t[:, :, 0:2], 0.0)
                    nc.sync.dma_start(x_t[:, :, 2:W], xT[:, :, 0:hi])
                    nc.vector.memset(x8_t[:, :, 0:2], 0.0)
                    nc.sync.dma_start(x8_t[:, :, 2:W], x8r[:, :, 0:hi])
                elif hi > N:
                    nc.vector.memset(x_t[:, :, W - 2:W], 0.0)
                    nc.sync.dma_start(x_t[:, :, 0:W - 2], xT[:, :, lo:N])
                    nc.vector.memset(x8_t[:, :, W - 2:W], 0.0)
                    nc.sync.dma_start(x8_t[:, :, 0:W - 2], x8r[:, :, lo:N])
                else:
                    nc.sync.dma_start(x_t, xT[:, :, lo:hi])
                    nc.sync.dma_start(x8_t[:, :, 0:W], x8r[:, :, lo:hi])
                xc_tiles[it] = (x_t, x8_t)

            load_xc(0)

            # ------------ Phase B: KV accumulation (fp8 DR, SBUF src) ------------
            with ExitStack() as phb:
                psb = phb.enter_context(tc.tile_pool(name="psB", bufs=1, space="PSUM"))
                for eh in range(D // 512):
                    hs = slice(eh * 512, (eh + 1) * 512)
                    kv_ps = [psb.tile([P, 512], F32, tag=f"kvps{d}", name=f"kvps{d}_{eh}")
                             for d in range(DO)]
                    for chp in range(0, TOTCH, 2):
                        for dc in range(DO):
                            nc.tensor.matmul(kv_ps[dc],
                                             kp_full[:, chp:chp + 2, dc * P:(dc + 1) * P],
                                             v_full[:, chp:chp + 2, hs],
                                             start=(chp == 0),
                                             stop=(chp == TOTCH - 2), perf_mode=DR)
                    for dc in range(DO):
                        nc.scalar.activation(kv_sb[:, dc, hs], kv_ps[dc], AF.Copy,
                                             scale=KV_SCALE)

        # ---------------- Phase C: conv'' + local MLP + token mixer ----------------
        with phC as ph:
            pipe = ph.enter_context(tc.tile_pool(name="pipeC", bufs=2))
            mid = ph.enter_context(tc.tile_pool(name="midC", bufs=1))
            sm = ph.enter_context(tc.tile_pool(name="smC", bufs=1))
            ps = ph.enter_context(tc.tile_pool(name="psC", bufs=2, space="PSUM"))
            pst = ph.enter_context(tc.tile_pool(name="pstC", bufs=1, space="PSUM"))

            def c_front(it):
                n0 = it * NT
                x_t, x8_t = xc_tiles.pop(it)
                acc = acc_full[:, :, n0:n0 + NT]
                # diffusion dwconv'' center tap + bias on ACT (side taps join
                # the c_back PSUM group as diagonal matmuls)
                for o in range(DO):
                    nc.scalar.activation(acc[:, o, :], x_t[:, o, 2:NT + 2],
                                         AF.Identity, bias=pp[:, o, CB:CB + 1],
                                         scale=pp[:, o, C1i:C1i + 1])

                # local MLP first half (fp8 DoubleRow; w1 pre-scaled x16)
                h1_t = pipe.tile([P, DO, NT], BF16, tag="h1", name=f"h1_{it}")
                for dc in range(DO):
                    ps_h = ps.tile([P, NT], F32, tag="psh1", name=f"psh1_{it}_{dc}")
                    for kp in range(0, DO, 2):
                        nc.tensor.matmul(ps_h, w1_sb[:, kp:kp + 2, dc * P:(dc + 1) * P],
                                         x8_t[:, kp:kp + 2, 2:NT + 2],
                                         start=(kp == 0), stop=(kp == DO - 2),
                                         perf_mode=DR)
                    nc.scalar.activation(h1_t[:, dc, :], ps_h, AF.Gelu,
                                         bias=pp[:, dc, LUB1:LUB1 + 1],
                                         scale=1.0 / 16.0)

                # token mixer LN stats via fp8 DR on x8 (ones=1; /D at readout)
                sq_t = mid.tile([P, DO, W], FP8, tag="sq8", name=f"sq_{it}")
                nc.scalar.activation(sq_t, x8_t[:, :, 0:W], AF.Square)
                ps_m = pst.tile([P, W], F32, tag="psm", name=f"psm_{it}")
                stats_mm8(ps_m, x8_t[:, :, 0:W], W)
                ps_s = pst.tile([P, W], F32, tag="pss", name=f"pss_{it}")
                stats_mm8(ps_s, sq_t, W)
                m_sb = sm.tile([P, W], BF16, tag="msb", name=f"msb_{it}")
                nc.scalar.activation(m_sb, ps_m, AF.Copy, scale=1.0 / D)
                var = sm.tile([P, W], F32, tag="var", name=f"var_{it}")
                nc.scalar.activation(var, ps_m, AF.Square, scale=1.0 / D)
                nc.vector.scalar_tensor_tensor(var, ps_s, 1.0 / D, var,
                                               OP.mult, OP.subtract)
                nc.scalar.activation(var, var, AF.Sqrt, bias=eps_ln[:, 0:1])
                nc.vector.reciprocal_approx_fast(out=var, in_=var)
                rstd = sm.tile([P, W], BF16, tag="rstd", name=f"rstd_{it}")
                nc.vector.tensor_copy(rstd, var)
                # xm = (x - m) * rstd  (tm gamma folded into conv1 taps)
                xm_t = mid.tile([P, DO, W], BF16, tag="tokC", name=f"xm_{it}")
                nc.vector.tensor_sub(xm_t, x_t,
                                     m_sb[:, None, :].broadcast_to([P, DO, W]))
                nc.vector.tensor_mul(xm_t, xm_t,
                                     rstd[:, None, :].broadcast_to([P, DO, W]))
                if use_tmb:
                    for o in range(DO):
                        nc.vector.tensor_scalar_add(xm_t[:, o, :], xm_t[:, o, :],
                                                    pp[:, o, TMB:TMB + 1])
                # conv1: t_s[k] = conv1(xm)[k+1], k in [0, W-2)
                t_t = mid.tile([P, DO, W - 2], BF16, tag="tokD", name=f"t_{it}")
                for o in range(DO):
                    nc.scalar.activation(t_t[:, o, :], xm_t[:, o, 1:W - 1],
                                         AF.Identity, bias=pp[:, o, TCB1:TCB1 + 1],
                                         scale=pp[:, o, T1:T1 + 1])
                for o in range(DO):
                    nc.vector.scalar_tensor_tensor(t_t[:, o, :], xm_t[:, o, 0:W - 2],
                                                   pp[:, o, T0:T0 + 1],
                                                   t_t[:, o, :], OP.mult, OP.add)
                for o in range(DO):
                    nc.vector.scalar_tensor_tensor(t_t[:, o, :], xm_t[:, o, 2:W],
                                                   pp[:, o, T2:T2 + 1],
                                                   t_t[:, o, :], OP.mult, OP.add)
                t2_t = pipe.tile([P, DO, W - 2], FP8, tag="t2", name=f"t2_{it}")
                nc.scalar.activation(t2_t, t_t, AF.Gelu)
                if it == 0:
                    nc.vector.memset(t2_t[:, :, 0:1], 0.0)
                if it == NTILES - 1:
                    nc.vector.memset(t2_t[:, :, W - 3:W - 2], 0.0)
                return x_t, h1_t, t2_t

            def c_back(it, tiles):
                n0 = it * NT
                x_t, h1_t, t2_t = tiles
                for dc in range(DO):
                    ps_h = ps.tile([P, NT], F32, tag="psh2", name=f"psh2_{it}_{dc}")
                    for kc in range(DO):
                        nc.tensor.matmul(ps_h, w2_sb[:, kc, dc * P:(dc + 1) * P],
                                         h1_t[:, kc, :],
                                         start=(kc == 0), stop=False)
                    for tap in range(3):
                        nc.tensor.matmul(ps_h, diags[:, tap, dc, :],
                                         t2_t[:, dc, tap:NT + tap],
                                         start=False, stop=False)
                    # diffusion conv side taps as diagonal matmuls
                    nc.tensor.matmul(ps_h, diagsb[:, 0, dc, :],
                                     x_t[:, dc, 1:NT + 1], start=False, stop=False)
                    nc.tensor.matmul(ps_h, diagsb[:, 1, dc, :],
                                     x_t[:, dc, 3:NT + 3], start=False, stop=True)
                    nc.vector.tensor_add(acc_full[:, dc, n0:n0 + NT],
                                         acc_full[:, dc, n0:n0 + NT], ps_h)

            pend = {0: c_front(0)}
            for it in range(NTILES):
                if it + 1 < NTILES:
                    load_xc(it + 1)
                    pend[it + 1] = c_front(it + 1)
                c_back(it, pend.pop(it))

        # ---------------- Phase D: attention + LN1 + FFN + LN2 ----------------
        with ExitStack() as ph:
            ioD = ph.enter_context(tc.tile_pool(name="ioD", bufs=2))
            qp_tiles = {}

            def load_qp(it):
                t = ioD.tile([P, DO, NT], FP8, tag="qpD", name=f"qp_{it}")
                nc.sync.dma_start(t, qp_r[:, :, it * NT:(it + 1) * NT])
                qp_tiles[it] = t

            # qp(0) queued ahead of the FFN weight DMAs so the first
            # numerator matmuls don't wait for 4MB of weights
            load_qp(0)
            wpoolD = ph.enter_context(tc.tile_pool(name="wD", bufs=1))
            f1_sb = wpoolD.tile([P, DO, D], BF16, tag="f1")
            nc.sync.dma_start(f1_sb, wr["f1T"])
            f2_sb = wpoolD.tile([P, DO, D], BF16, tag="f2")
            nc.sync.dma_start(f2_sb, wr["f2T"])
            mid = ph.enter_context(tc.tile_pool(name="midD", bufs=1))
            sm = ph.enter_context(tc.tile_pool(name="smD", bufs=2))
            ps = ph.enter_context(tc.tile_pool(name="psD", bufs=2, space="PSUM"))
            psf_pool = ph.enter_context(tc.tile_pool(name="psfD", bufs=3, space="PSUM"))
            pst = ph.enter_context(tc.tile_pool(name="pstD", bufs=1, space="PSUM"))

            def d_front_a(it):
                """norm row, C1/norm fold, numerator halves 0-3."""
                n0 = it * NT
                if it not in qp_tiles:
                    load_qp(it)
                qp_t = qp_tiles.pop(it)
                if it + 1 < NTILES:
                    load_qp(it + 1)
                acc_t = acc_full[:, :, n0:n0 + NT]
                ps_n = pst.tile([P, NT], F32, tag="psrep", name=f"psn_{it}")
                for kc in range(DO):
                    nc.tensor.matmul(ps_n[0:1, :], ksum_sb[:, kc, :],
                                     qp_t[:, kc, :],
                                     start=(kc == 0), stop=(kc == DO - 1))
                nr = sm.tile([1, NT], F32, tag="nrD", name=f"nr_{it}")
                nc.vector.tensor_scalar_add(nr, ps_n[0:1, :], 1e-6 * KSUM_SCALE)
                rr = sm.tile([1, NT], F32, tag="rrD", name=f"rr_{it}")
                nc.vector.reciprocal_approx_fast(out=rr, in_=nr)
                rrb = sm.tile([1, NT], BF16, tag="rrbD", name=f"rrb_{it}")
                nc.vector.tensor_copy(rrb, rr)
                # rep = C1*KSUM_SCALE * (1/(norm*KSUM_SCALE)) = C1/norm
                ps_rep = pst.tile([P, NT], F32, tag="psrep", name=f"psrep_{it}")
                nc.tensor.matmul(ps_rep, repc_row[0:1, :], rrb, start=True,
                                 stop=True)
                rep_sb = mid.tile([P, NT], BF16, tag="repsb", name=f"rep_{it}")
                nc.scalar.activation(rep_sb, ps_rep, AF.Copy)
                nc.vector.tensor_mul(qp_t, qp_t,
                                     rep_sb[:, None, :].broadcast_to([P, DO, NT]))
                for ec in range(DO // 2):
                    ps_u = ps.tile([P, NT], F32, tag="psnum", name=f"psnum_{it}_{ec}")
                    for kp in range(0, DO, 2):
                        nc.tensor.matmul(ps_u, kv_sb[:, kp:kp + 2, ec * P:(ec + 1) * P],
                                         qp_t[:, kp:kp + 2, :],
                                         start=(kp == 0), stop=(kp == DO - 2),
                                         perf_mode=DR)
                    nc.vector.scalar_tensor_tensor(acc_t[:, ec, :], ps_u,
                                                   1.0 / (KV_SCALE * C1),
                                                   acc_t[:, ec, :], OP.mult, OP.add)
                return qp_t, acc_t

            def d_front_b(it, T):
                qp_t, acc_t = T
                for ec in range(DO // 2, DO):
                    ps_u = ps.tile([P, NT], F32, tag="psnum", name=f"psnum_{it}_{ec}")
                    for kp in range(0, DO, 2):
                        nc.tensor.matmul(ps_u, kv_sb[:, kp:kp + 2, ec * P:(ec + 1) * P],
                                         qp_t[:, kp:kp + 2, :],
                                         start=(kp == 0), stop=(kp == DO - 2),
                                         perf_mode=DR)
                    nc.vector.scalar_tensor_tensor(acc_t[:, ec, :], ps_u,
                                                   1.0 / (KV_SCALE * C1),
                                                   acc_t[:, ec, :], OP.mult, OP.add)
                return acc_t

            def d_mid(it, acc_t):
                """LN1 stats + apply -> y1 (bf16; n1 gamma folded into f1)."""
                acc_bf = mid.tile([P, DO, NT], BF16, tag="accbf", name=f"accbf_{it}")
                nc.scalar.activation(acc_bf, acc_t, AF.Copy)
                sq_t = mid.tile([P, DO, NT], FP8, tag="sqD8", name=f"sqD_{it}")
                nc.scalar.activation(sq_t, acc_t, AF.Square)
                ps_m1 = pst.tile([P, NT], F32, tag="psm1", name=f"psm1_{it}")
                stats_mm(ps_m1, onesD_bf, acc_bf, NT)
                ps_s1 = pst.tile([P, NT], F32, tag="pss1", name=f"pss1_{it}")
                stats_mm8(ps_s1, sq_t, NT)
                m1_sb = sm.tile([P, NT], BF16, tag="m1sb", name=f"m1_{it}")
                nc.scalar.activation(m1_sb, ps_m1, AF.Copy)
                var1 = sm.tile([P, NT], F32, tag="varD", name=f"var1_{it}")
                nc.scalar.activation(var1, ps_m1, AF.Square)
                nc.vector.scalar_tensor_tensor(var1, ps_s1, 1.0 / D, var1,
                                               OP.mult, OP.subtract)
                nc.scalar.activation(var1, var1, AF.Sqrt, bias=eps_ln[:, 0:1])
                nc.vector.reciprocal_approx_fast(out=var1, in_=var1)
                rstd1 = sm.tile([P, NT], BF16, tag="rstdb", name=f"rstdb_{it}")
                nc.vector.tensor_copy(rstd1, var1)
                y1_t = mid.tile([P, DO, NT], BF16, tag="y1", name=f"y1_{it}")
                nc.vector.tensor_sub(y1_t, acc_bf,
                                     m1_sb[:, None, :].broadcast_to([P, DO, NT]))
                nc.vector.tensor_mul(y1_t, y1_t,
                                     rstd1[:, None, :].broadcast_to([P, DO, NT]))
                if use_n1b:
                    for o in range(DO):
                        nc.vector.tensor_scalar_add(y1_t[:, o, :], y1_t[:, o, :],
                                                    pp[:, o, N1B:N1B + 1])
                return y1_t

            def d_ffn(it, y1_t):
                f1h_t = mid.tile([P, DO, NT], BF16, tag="f1h", name=f"f1h_{it}")
                for dc in range(DO):
                    ps_f = psf_pool.tile([P, NT], F32, tag="psf",
                                         name=f"psf1_{it}_{dc}")
                    for kc in range(DO):
                        nc.tensor.matmul(ps_f, f1_sb[:, kc, dc * P:(dc + 1) * P],
                                         y1_t[:, kc, :],
                                         start=(kc == 0), stop=(kc == DO - 1))
                    nc.scalar.activation(f1h_t[:, dc, :], ps_f, AF.Gelu,
                                         bias=pp[:, dc, FFB1:FFB1 + 1])
                y2_t = mid.tile([P, DO, NT], BF16, tag="y2", name=f"y2_{it}")
                for dc in range(DO):
                    ps_f = psf_pool.tile([P, NT], F32, tag="psf",
                                         name=f"psf2_{it}_{dc}")
                    for kc in range(DO):
                        nc.tensor.matmul(ps_f, f2_sb[:, kc, dc * P:(dc + 1) * P],
                                         f1h_t[:, kc, :],
                                         start=(kc == 0), stop=(kc == DO - 1))
                    if use_n1g:
                        # y2 = y1*g + f2(h); (residual gamma must be re-applied)
                        nc.vector.scalar_tensor_tensor(y2_t[:, dc, :],
                                                       y1_t[:, dc, :],
                                                       pp[:, dc, N1G:N1G + 1],
                                                       ps_f, OP.mult, OP.add)
                    else:
                        nc.vector.scalar_tensor_tensor(y2_t[:, dc, :], ps_f,
                                                       pp[:, dc, FFB2:FFB2 + 1],
                                                       y1_t[:, dc, :], OP.add, OP.add)
                return y2_t

            def d_back(it, y2_t, c0=0, cw=None):
                if cw is None:
                    cw = NT
                n0 = it * NT + c0
                y2s = y2_t[:, :, c0:c0 + cw]
                sq2_t = mid.tile([P, DO, NT], FP8, tag="sq28", name=f"sq2_{it}_{c0}")
                nc.scalar.activation(sq2_t[:, :, 0:cw], y2s, AF.Square)
                ps_m2 = pst.tile([P, NT], F32, tag="psm1", name=f"psm2_{it}_{c0}")
                stats_mm(ps_m2[:, 0:cw], onesD_bf, y2s, cw)
                ps_s2 = pst.tile([P, NT], F32, tag="pss1", name=f"pss2_{it}_{c0}")
                stats_mm8(ps_s2[:, 0:cw], sq2_t[:, :, 0:cw], cw)
                m2_sb = sm.tile([P, NT], BF16, tag="m2sb", name=f"m2_{it}_{c0}")
                nc.scalar.activation(m2_sb[:, 0:cw], ps_m2[:, 0:cw], AF.Copy)
                var2 = sm.tile([P, NT], F32, tag="varD", name=f"var2_{it}_{c0}")
                nc.scalar.activation(var2[:, 0:cw], ps_m2[:, 0:cw], AF.Square)
                nc.vector.scalar_tensor_tensor(var2[:, 0:cw], ps_s2[:, 0:cw],
                                               1.0 / D, var2[:, 0:cw],
                                               OP.mult, OP.subtract)
                nc.scalar.activation(var2[:, 0:cw], var2[:, 0:cw], AF.Sqrt,
                                     bias=eps_ln[:, 0:1])
                nc.vector.reciprocal_approx_fast(out=var2[:, 0:cw],
                                                 in_=var2[:, 0:cw])
                rstd2 = sm.tile([P, NT], BF16, tag="rstd2", name=f"rstd2_{it}_{c0}")
                nc.vector.tensor_copy(rstd2[:, 0:cw], var2[:, 0:cw])
                yo_t = mid.tile([P, DO, NT], BF16, tag="yo", name=f"yo_{it}_{c0}")
                yo = yo_t[:, :, 0:cw]
                nc.vector.tensor_sub(yo, y2s,
                                     m2_sb[:, None, 0:cw].broadcast_to([P, DO, cw]))
                if use_n2g:
                    for o in range(DO):
                        nc.vector.scalar_tensor_tensor(yo[:, o, :], yo[:, o, :],
                                                       pp[:, o, N2G:N2G + 1],
                                                       rstd2[:, 0:cw],
                                                       OP.mult, OP.mult)
                else:
                    nc.vector.tensor_mul(yo, yo,
                                         rstd2[:, None, 0:cw].broadcast_to([P, DO, cw]))
                if use_n2b:
                    for o in range(DO):
                        nc.vector.tensor_scalar_add(yo[:, o, :], yo[:, o, :],
                                                    pp[:, o, N2B:N2B + 1])
                nc.sync.dma_start(yT[:, :, n0:n0 + cw], yo)

            # Pipeline: tile t+1's numerator halves run under tile t's LN/FFN,
            # and tile t+1's LN1 chain is issued before tile t's LN2 so the
            # FFN matmuls of t+1 can flow while t's LN2 drains on DVE/ACT.
            acc_cur = d_front_b(0, d_front_a(0))
            y1_cur = d_mid(0, acc_cur)
            for it in range(NTILES):
                Tnext = d_front_a(it + 1) if it + 1 < NTILES else None
                y2_cur = d_ffn(it, y1_cur)
                if Tnext is not None:
                    acc_next = d_front_b(it + 1, Tnext)
                    y1_cur = d_mid(it + 1, acc_next)
                    d_back(it, y2_cur)
                else:
                    # last tile: split LN2 so the final chain+DMA pipelines
                    d_back(it, y2_cur, 0, NT // 2)
                    d_back(it, y2_cur, NT // 2, NT // 2)

    nc.compile()
    return nc


def make_in_maps(inputs, n_cores=8):
    """Host-side preprocessing: fold constants, transpose, cast, shard."""
    x = np.asarray(inputs["x"], np.float32)
    B, N, D_ = x.shape
    dt = float(np.asarray(inputs["delta_t"]))

    def g(k):
        return np.asarray(inputs[k], np.float32)

    diff_w, diff_b = g("diff_w"), g("diff_b")
    tm_w1, tm_cb1 = g("tm_w1"), g("tm_cb1")
    tm_w2, tm_cb2 = g("tm_w2"), g("tm_cb2")
    tm_g = g("tm_g")
    n1_g, n2_g = g("n1_g"), g("n2_g")

    pp = np.zeros((P, DO, NPARAM), np.float32)

    def put(i, v):
        pp[:, :, i] = v.reshape(DO, P).T

    put(C0, dt * diff_w[:, 0, 0])
    put(C1i, dt * diff_w[:, 0, 1] + (1.0 - dt))
    put(C2, dt * diff_w[:, 0, 2])
    put(CB, dt * diff_b + g("lu_b2") + tm_cb2)
    # token-mixer gamma folded into the conv1 taps
    put(T0, tm_w1[:, 0, 0] * tm_g)
    put(T1, tm_w1[:, 0, 1] * tm_g)
    put(T2, tm_w1[:, 0, 2] * tm_g)
    put(TCB1, tm_cb1)
    put(U0, tm_w2[:, 0, 0])
    put(U1, tm_w2[:, 0, 1])
    put(U2, tm_w2[:, 0, 2])
    put(TMG, tm_g)
    put(TMB, g("tm_beta"))
    put(N1G, n1_g)
    put(N1B, g("n1_b"))
    put(N2G, n2_g)
    put(N2B, g("n2_b"))
    put(LUB1, g("lu_b1"))
    put(FFB1, g("ff_b1"))
    put(FFB2, g("ff_b2"))

    # diags (fp8): token-mixer conv2 taps; diagsb (bf16): diffusion side taps
    diags = np.zeros((P, 3, DO, P), np.float32)
    diagsb = np.zeros((P, 2, DO, P), np.float32)
    idx = np.arange(P)
    for tap in range(3):
        for dc in range(DO):
            diags[idx, tap, dc, idx] = tm_w2[dc * P + idx, 0, tap]
    for dc in range(DO):
        diagsb[idx, 0, dc, idx] = dt * diff_w[dc * P + idx, 0, 0]
        diagsb[idx, 1, dc, idx] = dt * diff_w[dc * P + idx, 0, 2]
    diags = np.clip(diags, -240, 240).astype(FP8_NP)
    diagsb = diagsb.astype(BF16_NP)

    rows = np.zeros((1, 3 * D), np.float32)
    rows[0, 0:D] = g("bq")
    rows[0, D:2 * D] = g("bk")
    rows[0, 2 * D:3 * D] = g("bv")
    rows = rows.astype(BF16_NP)

    use_n1g = bool(np.any(n1_g != 1.0))
    use_n2g = bool(np.any(n2_g != 1.0))

    wt = {}
    for name, key in (("w2T", "lu_w2"), ("f2T", "ff_w2")):
        wt[name] = np.ascontiguousarray(g(key).T).astype(BF16_NP)
    # n1 gamma folded into ff_w1 input rows (when gamma != 1)
    f1 = g("ff_w1")
    if use_n1g:
        f1 = f1 * n1_g[None, :]
    wt["f1T"] = np.ascontiguousarray(f1.T).astype(BF16_NP)
    # w1 shipped as fp8 pre-scaled x16 (consumer applies 1/16 via ACT scale)
    wt["w1T"] = np.ascontiguousarray(
        np.clip(g("lu_w1").T * 16.0, -240, 240)).astype(FP8_NP)
    for name, key in (("wqT", "wq"), ("wkT", "wk"), ("wvT", "wv")):
        wt[name] = np.ascontiguousarray(
            np.clip(g(key).T, -240, 240)).astype(FP8_NP)

    xT = np.ascontiguousarray(x.transpose(0, 2, 1)).astype(BF16_NP)
    x8 = np.clip(xT.astype(np.float32), -240, 240).astype(FP8_NP)

    flags = dict(
        use_bq=bool(np.any(g("bq"))),
        use_bk=bool(np.any(g("bk"))),
        use_bv=bool(np.any(g("bv"))),
        use_tmb=bool(np.any(g("tm_beta"))),
        use_n1b=bool(np.any(g("n1_b"))),
        use_n2b=bool(np.any(g("n2_b"))),
        use_n1g=use_n1g,
        use_n2g=use_n2g,
    )

    shared = {**wt, "pp": pp, "rows": rows, "diags": diags,
              "diagsb": diagsb}
    in_maps = [{**shared, "x_T": xT[b], "x8": x8[b]} for b in range(n_cores)]
    return in_maps, flags, (B, N)


_NC_CACHE = {}


def kernel(**inputs):
    in_maps, flags, (B, N) = make_in_maps(inputs)
    key = (N, tuple(sorted(flags.items())))
    if key not in _NC_CACHE:
        _NC_CACHE[key] = build_nc(N=N, NT=512, **flags)
    nc = _NC_CACHE[key]
    res = run_bass_kernel_spmd(nc, in_maps, list(range(B)))
    y = np.stack([res.results[b]["y_T"] for b in range(B)])
    return np.ascontiguousarray(y.transpose(0, 2, 1)).astype(np.float32)
